# revision 44
# baseline (speedup 1.0000x reference)
"""Trainium2 Bass kernel for nn_CSFAProV2 — full-input contract.

kernel(**inputs) takes the FULL unsharded inputs (B=32), shards the batch
across 8 NeuronCores (4 samples each, pure data parallel over axis 0 of
x1/x2, weights replicated), compiles+runs the Bass/Tile kernel via
run_bass_kernel_spmd, and concatenates the per-core outputs into the full
[32, 1024, 40, 40] result. Self-contained: the Bass kernel builder is
inlined below; only needs /opt/trn_rl_repo (concourse) + numpy/ml_dtypes.
"""

import sys

if '/opt/trn_rl_repo' not in sys.path:
    sys.path.insert(0, '/opt/trn_rl_repo')

import numpy as np

N_CORES = 8
B_FULL = 32
B_CORE = B_FULL // N_CORES

_cache = {}


def make_in_maps(inputs):
    wd = prep_weights(inputs)
    x1 = np.ascontiguousarray(np.asarray(inputs['x1'], np.float32))
    x2 = np.ascontiguousarray(np.asarray(inputs['x2'], np.float32))

    in_maps = []
    for c in range(N_CORES):
        m = dict(wd)
        m['x1'] = x1[c * B_CORE:(c + 1) * B_CORE]
        m['x2'] = x2[c * B_CORE:(c + 1) * B_CORE]
        in_maps.append(m)
    return in_maps


def kernel(**inputs):
    from concourse.bass_utils import run_bass_kernel_spmd

    if 'nc' not in _cache:
        _cache['nc'] = build_nc(B=B_CORE)
    nc = _cache['nc']

    in_maps = make_in_maps(inputs)
    res = run_bass_kernel_spmd(nc, in_maps, core_ids=list(range(N_CORES)))
    return np.concatenate([res.results[c]['out'] for c in range(N_CORES)], axis=0)


# ======================================================================
# Inlined Bass/Tile kernel builder (generated from kernel_lib.py)
# ======================================================================

"""Bass/Tile kernel builder for nn_CSFAProV2 (per-core, B batch samples).

q-conv (stride-2 3x3, bf16) -> cross-attention (transposed softmax;
DMA-transposed patch-major bf16 value slabs) -> conv3 3x3 fp32r + residual;
MGFAB branch (channel-MLP sigmoid gate, two 3x3 convs, 1x1 conv) in bf16.
Channel-on-partition layouts; convs = PE matmuls accumulating over
(ktile, dy, dx) on padded-spatial SBUF tiles.
"""

import math
import numpy as np
import concourse.bass as bass
import concourse.mybir as mybir
from concourse import bacc
from concourse.tile import TileContext
from concourse.alu_op_type import AluOpType

F32 = mybir.dt.float32
F32R = mybir.dt.float32r
BF16 = mybir.dt.bfloat16
F8E4 = mybir.dt.float8e4
AF = mybir.ActivationFunctionType

H = W = 40
WP = 42
PADPIX = WP * WP
CTB = PADPIX + 2 * 43  # fp8 attn block: 43 pad | 1764 | 43 pad
CHUNK_ROWS = 10
NCHUNK = H // CHUNK_ROWS
CHUNK_N = CHUNK_ROWS * W
VCOL = 128


def prep_weights(inp):
    import ml_dtypes
    bf16 = ml_dtypes.bfloat16
    d = {}

    def convT(w, kt_n, mt_n):  # [Cout, Cin, 3, 3] -> [kt, mt, 128, 9*128]
        x = np.asarray(w, np.float32).reshape(mt_n, 128, kt_n, 128, 9)
        x = x.transpose(2, 0, 3, 4, 1)
        return np.ascontiguousarray(x.reshape(kt_n, mt_n, 128, 9 * 128))

    d['qwT'] = convT(inp['q_w'], 4, 2).astype(bf16)
    # conv3 weights for fp8 DoubleRow: [pair g, mt, ki, (dydx, j, mo)]
    c3 = np.asarray(inp['c3_w'], np.float32).reshape(4, 128, 2, 2, 128, 9)
    c3 = c3.transpose(2, 0, 4, 5, 3, 1)  # g, mt, ki, dydx, j, mo
    d['c3w8'] = np.ascontiguousarray(c3.reshape(2, 4, 128, 9 * 2 * 128)).astype(
        ml_dtypes.float8_e4m3)
    d['sa1wT'] = convT(inp['sa1_w'], 2, 2).astype(bf16)
    d['sa2wT'] = convT(inp['sa2_w'], 2, 2).astype(bf16)
    w2 = np.asarray(inp['conv2_w'], np.float32)[:, :, 0, 0]
    d['c2wT'] = np.ascontiguousarray(
        w2.reshape(4, 128, 6, 128).transpose(2, 0, 3, 1)).astype(bf16)
    # the patch mean's /16 is folded into the key projection
    kw = np.asarray(inp['key_w'], np.float32) * 0.0625
    d['keywT'] = np.ascontiguousarray(
        kw.reshape(2, 128, 4, 128).transpose(2, 0, 3, 1))
    w1 = np.asarray(inp['ca_w1'], np.float32)
    d['caw1T'] = np.ascontiguousarray(
        w1.reshape(64, 2, 128).transpose(1, 2, 0)).astype(bf16)
    w2c = np.asarray(inp['ca_w2'], np.float32)
    d['caw2T'] = np.ascontiguousarray(
        w2c.reshape(2, 128, 64).transpose(2, 0, 1)).astype(bf16)
    d['zeros128'] = np.zeros((128, 128), np.float32)
    for nm, key in [('qs', 'q_s'), ('qb', 'q_b'), ('c3s', 'c3_s'), ('c3b', 'c3_b'),
                    ('sa1s', 'sa1_s'), ('sa1b', 'sa1_b'), ('sa2s', 'sa2_s'),
                    ('sa2b', 'sa2_b'), ('c2s', 'conv2_s'), ('c2b', 'conv2_b'),
                    ('cab1', 'ca_b1'), ('cab2', 'ca_b2')]:
        d[nm] = np.ascontiguousarray(np.asarray(inp[key], np.float32))
    return d


def build_nc(B=4, debug_taps=()):
    nc = bacc.Bacc(None)
    x1 = nc.dram_tensor("x1", [B, 512, 20, 20], F32, kind="ExternalInput")
    x2 = nc.dram_tensor("x2", [B, 512, 40, 40], F32, kind="ExternalInput")
    w = {}
    w['qwT'] = nc.dram_tensor("qwT", [4, 2, 128, 9 * 128], BF16, kind="ExternalInput")
    w['c3w8'] = nc.dram_tensor("c3w8", [2, 4, 128, 9 * 2 * 128], F8E4, kind="ExternalInput")
    w['sa1wT'] = nc.dram_tensor("sa1wT", [2, 2, 128, 9 * 128], BF16, kind="ExternalInput")
    w['sa2wT'] = nc.dram_tensor("sa2wT", [2, 2, 128, 9 * 128], BF16, kind="ExternalInput")
    w['c2wT'] = nc.dram_tensor("c2wT", [6, 4, 128, 128], BF16, kind="ExternalInput")
    w['keywT'] = nc.dram_tensor("keywT", [4, 2, 128, 128], F32R, kind="ExternalInput")
    w['caw1T'] = nc.dram_tensor("caw1T", [2, 128, 64], BF16, kind="ExternalInput")
    w['caw2T'] = nc.dram_tensor("caw2T", [64, 2, 128], BF16, kind="ExternalInput")
    for nm, n in [('qs', 256), ('qb', 256), ('c3s', 512), ('c3b', 512),
                  ('sa1s', 256), ('sa1b', 256), ('sa2s', 256), ('sa2b', 256),
                  ('c2s', 512), ('c2b', 512), ('cab1', 64), ('cab2', 256)]:
        w[nm] = nc.dram_tensor(nm, [n], F32, kind="ExternalInput")
    w['zeros128'] = nc.dram_tensor("zeros128", [128, 128], F32R, kind="ExternalInput")
    out = nc.dram_tensor("out", [B, 1024, 40, 40], F32, kind="ExternalOutput")

    taps = {}
    if 'q' in debug_taps:
        taps['q'] = nc.dram_tensor("tap_q", [2, 128, B * 100], F32, kind="ExternalOutput")
    if 'k' in debug_taps:
        taps['k'] = nc.dram_tensor("tap_k", [2, 128, B * 100], F32, kind="ExternalOutput")
    if 'attn' in debug_taps:
        taps['attn'] = nc.dram_tensor("tap_attn", [B, 4, 128, PADPIX], F32, kind="ExternalOutput")
    if 'gate' in debug_taps:
        taps['gate'] = nc.dram_tensor("tap_gate", [B, 2, 128, 1600], F32, kind="ExternalOutput")
    if 'a2' in debug_taps:
        taps['a2'] = nc.dram_tensor("tap_a2", [B, 2, 128, 1600], F32, kind="ExternalOutput")

    with TileContext(nc) as tc:
        _emit(nc, tc, B, x1, x2, w, out, taps)
    nc.finalize()
    return nc


def _apron_memset(nc, t):
    nc.gpsimd.memset(t[:, 0:WP], 0.0)
    nc.gpsimd.memset(t[:, 41 * WP:42 * WP], 0.0)
    g = t[:].rearrange("p (y x) -> p y x", x=WP)
    nc.gpsimd.memset(g[:, 1:41, 0:1], 0.0)
    nc.gpsimd.memset(g[:, 1:41, 41:42], 0.0)


def _apron_zero_dma(nc, t, zdram):
    # f32r tiles can't be memset; DMA zeros from DRAM (same dtype, no cast)
    g = t[:].rearrange("p (y x) -> p y x", x=WP)
    rows = g[:, 0:42:41, :]            # rows 0 and 41
    nc.sync.dma_start(out=rows, in_=zdram.ap()[:, 0:84].rearrange("p (a b) -> p a b", a=2))
    nc.sync.dma_start(out=g[:, 1:41, 0:1], in_=zdram.ap()[:, 0:40])
    nc.sync.dma_start(out=g[:, 1:41, 41:42], in_=zdram.ap()[:, 0:40])


def _emit(nc, tc, B, x1, x2, w, out, taps):
    import contextlib
    ctx = contextlib.ExitStack()
    with ctx:
        from concourse import masks
        mp = ctx.enter_context(tc.tile_pool(name="main", bufs=1))
        psC = ctx.enter_context(tc.tile_pool(name="psC", bufs=2, space="PSUM"))
        psY = ctx.enter_context(tc.tile_pool(name="psY", bufs=4, space="PSUM"))
        psT = ctx.enter_context(tc.tile_pool(name="psT", bufs=2, space="PSUM"))

        ones_bf = mp.tile([128, 1], BF16, tag="ones")
        nc.gpsimd.memset(ones_bf[:], 1.0)
        ident = mp.tile([128, 128], BF16, tag="ident")
        masks.make_identity(nc, ident[:])

        def load_vec(name, n):
            p = min(n, 128)
            t = mp.tile([128, max(n // 128, 1)], F32, tag=f"vec_{name}")
            nc.sync.dma_start(out=t[0:p, 0:max(n // 128, 1)],
                              in_=w[name].ap().rearrange("(a p) -> p a", p=p))
            return t
        vs = {nm: load_vec(nm, n) for nm, n in
              [('qs', 256), ('qb', 256), ('c3s', 512), ('c3b', 512),
               ('sa1s', 256), ('sa1b', 256), ('sa2s', 256), ('sa2b', 256),
               ('c2s', 512), ('c2b', 512), ('cab1', 64), ('cab2', 256)]}

        # resident small weights
        c2_sb = mp.tile([128, 6 * 512], BF16, tag="c2w")
        for kt in range(6):
            nc.sync.dma_start(
                out=c2_sb[:, kt * 512:(kt + 1) * 512].rearrange("p (m c) -> p m c", m=4),
                in_=w['c2wT'][kt].rearrange("m p c -> p m c"))
        keyw_sb = mp.tile([128, 8 * 128], F32R, tag="keyw")
        for kt in range(4):
            nc.sync.dma_start(
                out=keyw_sb[:, kt * 256:(kt + 1) * 256].rearrange("p (m c) -> p m c", m=2),
                in_=w['keywT'][kt].rearrange("m p c -> p m c"))
        caw1_sb = mp.tile([128, 128], BF16, tag="caw1")
        for kt in range(2):
            nc.sync.dma_start(out=caw1_sb[:, kt * 64:(kt + 1) * 64], in_=w['caw1T'][kt])
        caw2_sb = mp.tile([64, 256], BF16, tag="caw2")
        nc.sync.dma_start(out=caw2_sb[:], in_=w['caw2T'].rearrange("p m c -> p (m c)"))
        # resident fp8 conv3 weights: 8 slabs of [128, 9*2*128] (g*4+mt)
        c3w8_sb = mp.tile([128, 8 * 2304], F8E4, tag="c3w8")
        for g in range(2):
            for mt in range(4):
                nc.sync.dma_start(
                    out=c3w8_sb[:, (g * 4 + mt) * 2304:(g * 4 + mt + 1) * 2304],
                    in_=w['c3w8'][g, mt])

        # x1: load + zero-padded bf16 [128, (s, 22, 22)]
        x1pad = []
        for ct in range(4):
            t = mp.tile([128, B * 484], BF16, tag=f"x1pad{ct}")
            nc.gpsimd.memset(t[:], 0.0)
            raw = mp.tile([128, B * 400], F32, tag="x2", bufs=3)
            src = x1.ap()[:, ct * 128:(ct + 1) * 128].rearrange("s p y x -> p s (y x)")
            nc.sync.dma_start(out=raw[:].rearrange("p (s a) -> p s a", s=B), in_=src)
            dst = t[:].rearrange("p (s y x) -> p s y x", s=B, x=22)[:, :, 1:21, 1:21]
            rawv = raw[:].rearrange("p (s y x) -> p s y x", s=B, x=20)
            hb = B // 2
            nc.gpsimd.tensor_copy(dst[:, 0:hb], rawv[:, 0:hb])
            nc.vector.tensor_copy(dst[:, hb:B], rawv[:, hb:B])
            x1pad.append(t)

        def up_ap(ct, s, chunk):
            y0h = chunk * CHUNK_ROWS // 2
            base = x1pad[ct][:].rearrange("p (ss a) -> p ss a", ss=B)[:, s]
            base = base.rearrange("p (y x) -> p y x", x=22)[:, 1:21, 1:21]
            up = base.unsqueeze(3).broadcast_to([128, 20, 20, 2])
            return up[:, y0h:y0h + 5]

        # ---------- q conv ----------
        qslabs = {}
        for mt in range(2):
            for kt in range(4):
                qbf = mp.tile([128, 1152], BF16, tag="wsbf", bufs=5)
                nc.sync.dma_start(out=qbf[:], in_=w['qwT'][kt, mt])
                qslabs[(kt, mt)] = qbf
        q_sb = mp.tile([128, 2 * B * 100], BF16, tag="qsb")
        for mt in range(2):
            ps0 = psC.tile([128, 512], F32, tag="cps", name="cps")
            ps = ps0[:, 0:B * 100]
            first = True
            for kt in range(4):
                base = x1pad[kt][:].rearrange("p (s y x) -> p s y x", s=B, x=22)
                for dy in range(3):
                    for dx in range(3):
                        rhs = base[:, :, dy:dy + 20:2, dx:dx + 20:2]
                        nc.tensor.matmul(
                            ps, qslabs[(kt, mt)][:, (dy * 3 + dx) * 128:(dy * 3 + dx + 1) * 128],
                            rhs, start=first, stop=(kt == 3 and dy == 2 and dx == 2))
                        first = False
            nc.scalar.activation(q_sb[:, mt * B * 100:(mt + 1) * B * 100], ps, AF.Silu,
                                 bias=vs['qb'][:, mt:mt + 1], scale=vs['qs'][:, mt:mt + 1])
        if 'q' in taps:
            for mt in range(2):
                qf = mp.tile([128, B * 100], F32, tag="tapq", bufs=1)
                nc.vector.tensor_copy(qf[:], q_sb[:, mt * B * 100:(mt + 1) * B * 100])
                nc.sync.dma_start(out=taps['q'][mt], in_=qf[:])

        k_sb = mp.tile([128, 2 * B * 100], BF16, tag="ksb")
        SCALE = 1.0 / math.sqrt(32)

        # ---------- per-sample pipeline ----------
        # persistent big tensors: aprons/garbage zeroed once, interiors
        # rewritten per sample (pool-slot rotation would re-zero every pass
        # and flood the DMA queues with tiny descriptors)
        # attn (bf16, no apron — residual only) is allocated per-sample with
        # 2 rotating slots for cross-sample overlap
        # fp8 copy of attn for conv3 DoubleRow rhs (aprons + pads zeroed once);
        # two slots so sample s+1's cast overlaps sample s's conv3 reads
        attn8 = []
        for sl in range(2):
            a8 = mp.tile([128, 4 * CTB], F8E4, tag=f"attn8_{sl}", bufs=1)
            nc.gpsimd.memset(a8[:], 0.0)
            attn8.append(a8)
        xca_slots = []
        xsa1 = []
        a2 = []
        for i in range(2):
            for sl in range(2):
                t = mp.tile([128, PADPIX], BF16, tag=f"xca{i}_{sl}", name="t", bufs=1)
                _apron_memset(nc, t)
                xca_slots.append(t)
            t2 = mp.tile([128, PADPIX], BF16, tag=f"xsa{i}", name="t2", bufs=1)
            _apron_memset(nc, t2)
            xsa1.append(t2)
            a2t = mp.tile([128, 1600], BF16, tag=f"a2_{i}", name="a2t", bufs=1)
            a2.append(a2t)

        for s in range(B):
            attn8v = attn8[s % 2][:].rearrange("p (c n) -> p c n", c=4)
            attn = [mp.tile([128, 1600], BF16, tag=f"attn{ct}", name="at", bufs=2)
                    for ct in range(4)]
            xca = [xca_slots[0 * 2 + s % 2], xca_slots[1 * 2 + s % 2]]
            # ---- MGFAB: CA gate, phase-split to batch relu/sigmoid tables ----
            hsbs = []
            for chunk in range(NCHUNK):
                # hps layout: col = r*200 + y*40 + x  (pixel (2y+r)*40+x)
                hps = psC.tile([64, 512], F32, tag="cps", name="cps")
                for r in range(2):
                    for i in range(2):
                        nc.tensor.matmul(hps[:, r * 200:(r + 1) * 200],
                                         caw1_sb[:, i * 64:(i + 1) * 64],
                                         up_ap(2 + i, s, chunk),
                                         start=(i == 0 and r == 0),
                                         stop=(i == 1 and r == 1))
                hsb = mp.tile([64, CHUNK_N], BF16, tag="hsb", bufs=4)
                hview = hsb[:].rearrange("p (y r x) -> p y r x", y=5, r=2)
                pview = hps[:, 0:400].rearrange("p (r y x) -> p y r x", r=2, y=5)
                nc.scalar.activation(hview, pview, AF.Relu, bias=vs['cab1'][0:64, 0:1])
                hsbs.append(hsb)
            for chunk in range(NCHUNK):
                gt = mp.tile([128, 2 * CHUNK_N], BF16, tag="gate", bufs=3)
                for mt in range(2):
                    gps = psC.tile([128, 512], F32, tag="cps", name="cps")
                    nc.tensor.matmul(gps[:, 0:400], caw2_sb[0:64, mt * 128:(mt + 1) * 128],
                                     hsbs[chunk][:])
                    nc.scalar.activation(gt[:, mt * CHUNK_N:(mt + 1) * CHUNK_N], gps[:, 0:400],
                                         AF.Sigmoid, bias=vs['cab2'][:, mt:mt + 1])
                y0 = chunk * CHUNK_ROWS
                for i in range(2):
                    for r in range(2):
                        dst = xca[i][:].rearrange("p (y x) -> p y x", x=WP)
                        dst = dst[:, 1 + y0 + r:1 + y0 + 10:2, 1:41]
                        g = gt[:, i * CHUNK_N:(i + 1) * CHUNK_N]
                        g = g.rearrange("p (y x) -> p y x", x=40)[:, r::2]
                        nc.gpsimd.tensor_tensor(dst, up_ap(2 + i, s, chunk), g, AluOpType.mult)
            # ---- x2 pipeline: load / patch-sum / PE-transpose value slabs ----
            kp = []
            xts = []
            for ct in range(4):
                xt = mp.tile([128, 1600], F32, tag="x2", bufs=3)
                xts.append(xt)
                nc.sync.dma_start(
                    out=xt[:], in_=x2.ap()[s, ct * 128:(ct + 1) * 128].rearrange("p y x -> p (y x)"))
                vx = xt[:].rearrange("p (phy py pwx px) -> p phy pwx py px",
                                     phy=10, py=4, pwx=10, px=4)
                kpt = mp.tile([128, 100], F32R, tag="kp", bufs=5)
                with nc.allow_low_precision(reason="f32r is fp32-width"):
                    nc.vector.tensor_reduce(kpt[:].rearrange("p (a b) -> p a b", b=10),
                                            vx, mybir.AxisListType.XY, AluOpType.add)
                kp.append(kpt)
            # regroup xt -> patch-major bf16 slabs (vector/scalar split),
            # then PE-transpose the contiguous [128,100] slabs; 4 transposes
            # pack one PSUM bank, one copy moves them to vt
            def emit_regroup(ct, engine):
                xbf = mp.tile([128, 16 * 100], BF16, tag="x2bf", bufs=4)
                xbv = xbf[:].rearrange("p (py px c) -> p py px c", py=4, px=4)
                for py in range(4):
                    srcap = xts[ct][:].rearrange("p (phy py pwx px) -> p py phy pwx px",
                                                 phy=10, py=4, pwx=10, px=4)[:, py]
                    dstap = xbv[:, py].rearrange("p px (phy pwx) -> p phy pwx px", phy=10)
                    if engine == 'v':
                        nc.vector.tensor_copy(dstap, srcap)
                    else:
                        nc.scalar.copy(dstap, srcap)
                return xbf

            def emit_transpose(xbf):
                vt = mp.tile([128, 16 * 128], BF16, tag="v", bufs=5)
                for q4 in range(4):
                    pst = psT.tile([128, 512], BF16, tag="tp", name="tp")
                    for j in range(4):
                        pp = q4 * 4 + j
                        nc.tensor.transpose(pst[0:100, j * 128:(j + 1) * 128],
                                            xbf[:, pp * 100:(pp + 1) * 100], ident[:])
                    dst = vt[0:100, q4 * 512:(q4 + 1) * 512]
                    if q4 % 2 == 0:
                        nc.scalar.activation(dst, pst[0:100, :], AF.Copy)
                    else:
                        nc.vector.tensor_copy(dst, pst[0:100, :])
                return vt

            v_ct = []
            for ct in range(2):
                v_ct.append(emit_transpose(emit_regroup(ct, 'v')))

            # ---- k projection + scores (batched) + exp + 1/sum ----
            for mt in range(2):
                psk0 = psY.tile([128, 512], F32, tag="aps", name="aps")
                psk = psk0[:, 0:100]
                for kt in range(4):
                    nc.tensor.matmul(
                        psk, keyw_sb[:, (kt * 2 + mt) * 128:(kt * 2 + mt + 1) * 128],
                        kp[kt][:], start=(kt == 0), stop=(kt == 3))
                nc.scalar.copy(k_sb[:, (mt * B + s) * 100:(mt * B + s + 1) * 100],
                               psk)
            exps = []
            for h in range(8):
                emb_ct, emb_off = h // 4, (h % 4) * 32
                pssc0 = psY.tile([128, 512], F32, tag="aps", name="aps")
                pssc = pssc0[0:100, 0:100]
                lhs = k_sb[emb_off:emb_off + 32, (emb_ct * B + s) * 100:(emb_ct * B + s + 1) * 100]
                rhs = q_sb[emb_off:emb_off + 32, (emb_ct * B + s) * 100:(emb_ct * B + s + 1) * 100]
                nc.tensor.matmul(pssc, lhs, rhs, tile_position=(emb_off, 0))
                expT = mp.tile([100, 100], BF16, tag="expT", bufs=8)
                nc.scalar.activation(expT[:], pssc, AF.Exp, scale=SCALE)
                exps.append(expT)
            recs = []
            rbcs = []
            for half in range(2):
                pssum = psY.tile([128, 512], F32, tag="aps", name="aps")
                for j in range(4):
                    nc.tensor.matmul(pssum[0:1, j * 100:(j + 1) * 100],
                                     ones_bf[0:100, 0:1], exps[half * 4 + j][:],
                                     start=(j == 0), stop=(j == 3))
                recip = mp.tile([1, 400], F32, tag="recip", bufs=3)
                nc.vector.reciprocal(recip[:], pssum[0:1, 0:400])
                rbc = mp.tile([128, 400], F32, tag="rbc", bufs=2)
                nc.gpsimd.partition_broadcast(rbc[:], recip[:])
                rbcs.append(rbc)
            for h in range(8):
                recs.append((exps[h],
                             rbcs[h // 4][:, (h % 4) * 100:(h % 4) * 100 + 100]))

            v_ct.append(emit_transpose(emit_regroup(2, 's')))
            v_ct.append(emit_transpose(emit_regroup(3, 'v')))


            if 'gate' in taps:
                for i in range(2):
                    gf = mp.tile([128, 1600], F32, tag="tapg", bufs=1)
                    for chunk in range(NCHUNK):
                        nc.vector.tensor_copy(gf[:, chunk * 400:(chunk + 1) * 400],
                                              gates[chunk][:, i * CHUNK_N:(i + 1) * CHUNK_N])
                    nc.sync.dma_start(out=taps['gate'][s, i], in_=gf[:])
            # ---- SA conv1 ----
            def stream_sa(wt):
                slabs = {}
                for mt in range(2):
                    for kt in range(2):
                        tl = mp.tile([128, 1152], BF16, tag="wsbf", bufs=5)
                        nc.sync.dma_start(out=tl[:], in_=wt[kt, mt])
                        slabs[(kt, mt)] = tl
                return slabs

            def conv3x3(src_tiles, slabs, mt, kt_n, chunk):
                ps0 = psC.tile([128, 512], F32, tag="cps", name="cps")
                ps = ps0[:, 0:CHUNK_N]
                y0 = chunk * CHUNK_ROWS
                first = True
                for kt in range(kt_n):
                    base = src_tiles[kt][:].rearrange("p (y x) -> p y x", x=WP)
                    for dy in range(3):
                        for dx in range(3):
                            rhs = base[:, y0 + dy:y0 + dy + CHUNK_ROWS, dx:dx + 40]
                            lhsT = slabs[(kt, mt)][:, (dy * 3 + dx) * 128:(dy * 3 + dx + 1) * 128]
                            nc.tensor.matmul(ps, lhsT, rhs, start=first,
                                             stop=(kt == kt_n - 1 and dy == 2 and dx == 2))
                            first = False
                return ps

            # ---- attention apply interleaved with SA conv1 (keeps PE fed
            # while vector drains assembles); per-ct fp8 cast as soon as both
            # heads of a ct block are assembled ----
            sa1slabs = stream_sa(w['sa1wT'])
            sa1_jobs = [(mt, chunk) for mt in range(2) for chunk in range(NCHUNK)]
            attn_nop = [ct_t[:].rearrange("p (y x) -> p y x", x=40) for ct_t in attn]
            for h in range(8):
                expT, rbc = recs[h]
                o = (h % 2) * 64
                for py in range(4):
                    psy = psY.tile([128, 512], F32, tag="aps", name="aps")
                    for px in range(4):
                        pp = py * 4 + px
                        lhsT = v_ct[h // 2][0:100, pp * 128 + o: pp * 128 + o + 64]
                        nc.tensor.matmul(psy[o:o + 64, px * 100:px * 100 + 100], lhsT, expT[:],
                                         start=(px == 0), stop=(px == 3))
                    dstg = attn_nop[h // 2][o:o + 64, py:40:4, :]
                    dstg = dstg.rearrange("p a (pwx px) -> p a pwx px", px=4)
                    in0 = psy[o:o + 64, 0:400].rearrange("p (px phy pwx) -> p phy pwx px",
                                                         px=4, phy=10)
                    in1 = rbc[o:o + 64].rearrange("p (a b) -> p a b", b=10)
                    in1 = in1.unsqueeze(3).broadcast_to([64, 10, 10, 4])
                    nc.vector.scalar_tensor_tensor(dstg, in0, 0.0, in1,
                                                   AluOpType.bypass, AluOpType.mult)
                if h % 2 == 1:
                    ct = h // 2
                    dst8 = attn8v[:, ct, 43:43 + PADPIX].rearrange("p (y x) -> p y x", x=42)
                    nc.vector.tensor_copy(dst8[:, 1:41, 1:41], attn_nop[ct])
                # one SA1 (mt, chunk) group between heads keeps the PE busy
                mt, chunk = sa1_jobs[h]
                ps = conv3x3(xca, sa1slabs, mt, 2, chunk)[:, 0:CHUNK_N]
                y0 = chunk * CHUNK_ROWS
                dst = xsa1[mt][:].rearrange("p (y x) -> p y x", x=WP)[:, 1 + y0:11 + y0, 1:41]
                nc.scalar.activation(dst, ps.rearrange("p (a b) -> p a b", b=40), AF.Silu,
                                     bias=vs['sa1b'][:, mt:mt + 1], scale=vs['sa1s'][:, mt:mt + 1])

            # ---- SA conv2 + residual ----
            sa2slabs = stream_sa(w['sa2wT'])
            for mt in range(2):
                for chunk in range(NCHUNK):
                    ps = conv3x3(xsa1, sa2slabs, mt, 2, chunk)[:, 0:CHUNK_N]
                    y0 = chunk * CHUNK_ROWS
                    tsilu = mp.tile([128, CHUNK_N], F32, tag="silu", bufs=3)
                    nc.scalar.activation(tsilu[:], ps, AF.Silu,
                                         bias=vs['sa2b'][:, mt:mt + 1], scale=vs['sa2s'][:, mt:mt + 1])
                    xc = xca[mt][:].rearrange("p (y x) -> p y x", x=WP)[:, 1 + y0:11 + y0, 1:41]
                    nc.gpsimd.tensor_tensor(a2[mt][:, y0 * 40:(y0 + 10) * 40],
                                            tsilu[:].rearrange("p (a b) -> p a b", b=40),
                                            xc, AluOpType.add)
            if 'a2' in taps:
                for i in range(2):
                    af = mp.tile([128, 1600], F32, tag="tapg", bufs=1)
                    nc.vector.tensor_copy(af[:], a2[i][:])
                    nc.sync.dma_start(out=taps['a2'][s, i], in_=af[:])

            # ---- conv3 (fp8 DoubleRow) + residual -> x2_out ----
            for mt in range(4):
                for chunk in range(NCHUNK):
                    ps0 = psC.tile([128, 512], F32, tag="cps", name="cps")
                    ps = ps0[:, 0:420]
                    y0 = chunk * CHUNK_ROWS
                    first = True
                    for g in range(2):
                        wslab = c3w8_sb[:, (g * 4 + mt) * 2304:(g * 4 + mt + 1) * 2304] \
                            .rearrange("p (k two m) -> p k two m", two=2, m=128)
                        for dy in range(3):
                            for dx in range(3):
                                s0 = 42 + (y0 + dy) * 42 + dx
                                rhs = attn8v[:, 2 * g:2 * g + 2, s0:s0 + 420]
                                nc.tensor.matmul(ps, wslab[:, dy * 3 + dx], rhs,
                                                 start=first,
                                                 stop=(g == 1 and dy == 2 and dx == 2),
                                                 perf_mode=mybir.MatmulPerfMode.DoubleRow)
                                first = False
                    tsilu = mp.tile([128, CHUNK_N], F32, tag="silu", bufs=3)
                    psv = ps.rearrange("p (y x) -> p y x", x=42)[:, :, 1:41]
                    nc.scalar.activation(tsilu[:].rearrange("p (a b) -> p a b", b=40),
                                         psv, AF.Silu,
                                         bias=vs['c3b'][:, mt:mt + 1], scale=vs['c3s'][:, mt:mt + 1])
                    osb = mp.tile([128, CHUNK_N], F32, tag="osb", bufs=4)
                    at2 = attn_nop[mt][:, y0:y0 + 10, :]
                    nc.gpsimd.tensor_tensor(osb[:].rearrange("p (a b) -> p a b", b=40),
                                            tsilu[:].rearrange("p (a b) -> p a b", b=40),
                                            at2, AluOpType.add)
                    nc.gpsimd.dma_start(
                        out=out.ap()[s, 512 + mt * 128:512 + (mt + 1) * 128]
                            .rearrange("p y x -> p (y x)")[:, y0 * 40:(y0 + 10) * 40],
                        in_=osb[:])

            # ---- conv2 (1x1) -> x1_out ----
            for mt in range(4):
                for chunk in range(NCHUNK):
                    # ps layout: col = r*200 + y*40 + x  (pixel (2y+r)*40+x)
                    ps0 = psC.tile([128, 512], F32, tag="cps", name="cps")
                    ps = ps0[:, 0:CHUNK_N]
                    y0 = chunk * CHUNK_ROWS
                    first = True
                    for r in range(2):
                        for kt in range(4):
                            nc.tensor.matmul(
                                ps[:, r * 200:(r + 1) * 200],
                                c2_sb[:, (kt * 4 + mt) * 128:(kt * 4 + mt + 1) * 128],
                                up_ap(kt, s, chunk), start=first, stop=False)
                            first = False
                    for i in range(2):
                        kt = 4 + i
                        rhs = a2[i][:, y0 * 40:(y0 + 10) * 40]
                        rhs = rhs.rearrange("p (y r x) -> p r y x", y=5, r=2)
                        nc.tensor.matmul(ps, c2_sb[:, (kt * 4 + mt) * 128:(kt * 4 + mt + 1) * 128],
                                         rhs, start=False, stop=(i == 1))
                    osb = mp.tile([128, CHUNK_N], F32, tag="osb", bufs=4)
                    oview = osb[:].rearrange("p (y r x) -> p y r x", y=5, r=2)
                    pv = ps.rearrange("p (r y x) -> p y r x", r=2, y=5)
                    nc.scalar.activation(oview, pv, AF.Silu,
                                         bias=vs['c2b'][:, mt:mt + 1], scale=vs['c2s'][:, mt:mt + 1])
                    nc.scalar.dma_start(
                        out=out.ap()[s, mt * 128:(mt + 1) * 128]
                            .rearrange("p y x -> p (y x)")[:, y0 * 40:(y0 + 10) * 40],
                        in_=osb[:])



# revision 45
# speedup vs baseline: 1.0039x; 1.0039x over previous
"""Trainium2 Bass kernel for nn_CSFAProV2 — full-input contract.

kernel(**inputs) takes the FULL unsharded inputs (B=32), shards the batch
across 8 NeuronCores (4 samples each, pure data parallel over axis 0 of
x1/x2, weights replicated), compiles+runs the Bass/Tile kernel via
run_bass_kernel_spmd, and concatenates the per-core outputs into the full
[32, 1024, 40, 40] result. Self-contained: the Bass kernel builder is
inlined below; only needs /opt/trn_rl_repo (concourse) + numpy/ml_dtypes.
"""

import sys

if '/opt/trn_rl_repo' not in sys.path:
    sys.path.insert(0, '/opt/trn_rl_repo')

import numpy as np

N_CORES = 8
B_FULL = 32
B_CORE = B_FULL // N_CORES

_cache = {}


def make_in_maps(inputs):
    wd = prep_weights(inputs)
    x1 = np.ascontiguousarray(np.asarray(inputs['x1'], np.float32))
    x2 = np.ascontiguousarray(np.asarray(inputs['x2'], np.float32))

    in_maps = []
    for c in range(N_CORES):
        m = dict(wd)
        m['x1'] = x1[c * B_CORE:(c + 1) * B_CORE]
        m['x2'] = x2[c * B_CORE:(c + 1) * B_CORE]
        in_maps.append(m)
    return in_maps


def kernel(**inputs):
    from concourse.bass_utils import run_bass_kernel_spmd

    if 'nc' not in _cache:
        _cache['nc'] = build_nc(B=B_CORE)
    nc = _cache['nc']

    in_maps = make_in_maps(inputs)
    res = run_bass_kernel_spmd(nc, in_maps, core_ids=list(range(N_CORES)))
    return np.concatenate([res.results[c]['out'] for c in range(N_CORES)], axis=0)


# ======================================================================
# Inlined Bass/Tile kernel builder (generated from kernel_lib.py)
# ======================================================================

"""Bass/Tile kernel builder for nn_CSFAProV2 (per-core, B batch samples).

q-conv (stride-2 3x3, bf16) -> cross-attention (transposed softmax;
DMA-transposed patch-major bf16 value slabs) -> conv3 3x3 fp32r + residual;
MGFAB branch (channel-MLP sigmoid gate, two 3x3 convs, 1x1 conv) in bf16.
Channel-on-partition layouts; convs = PE matmuls accumulating over
(ktile, dy, dx) on padded-spatial SBUF tiles.
"""

import math
import numpy as np
import concourse.bass as bass
import concourse.mybir as mybir
from concourse import bacc
from concourse.tile import TileContext
from concourse.alu_op_type import AluOpType

F32 = mybir.dt.float32
F32R = mybir.dt.float32r
BF16 = mybir.dt.bfloat16
F8E4 = mybir.dt.float8e4
AF = mybir.ActivationFunctionType

H = W = 40
WP = 42
PADPIX = WP * WP
CTB = PADPIX + 2 * 43  # fp8 attn block: 43 pad | 1764 | 43 pad
CHUNK_ROWS = 10
NCHUNK = H // CHUNK_ROWS
CHUNK_N = CHUNK_ROWS * W
VCOL = 128


def prep_weights(inp):
    import ml_dtypes
    bf16 = ml_dtypes.bfloat16
    d = {}

    def convT(w, kt_n, mt_n):  # [Cout, Cin, 3, 3] -> [kt, mt, 128, 9*128]
        x = np.asarray(w, np.float32).reshape(mt_n, 128, kt_n, 128, 9)
        x = x.transpose(2, 0, 3, 4, 1)
        return np.ascontiguousarray(x.reshape(kt_n, mt_n, 128, 9 * 128))

    d['qwT'] = convT(inp['q_w'], 4, 2).astype(bf16)
    # conv3 weights for fp8 DoubleRow: [pair g, mt, ki, (dydx, j, mo)]
    c3 = np.asarray(inp['c3_w'], np.float32).reshape(4, 128, 2, 2, 128, 9)
    c3 = c3.transpose(2, 0, 4, 5, 3, 1)  # g, mt, ki, dydx, j, mo
    d['c3w8'] = np.ascontiguousarray(c3.reshape(2, 4, 128, 9 * 2 * 128)).astype(
        ml_dtypes.float8_e4m3)
    d['sa1wT'] = convT(inp['sa1_w'], 2, 2).astype(bf16)
    d['sa2wT'] = convT(inp['sa2_w'], 2, 2).astype(bf16)
    w2 = np.asarray(inp['conv2_w'], np.float32)[:, :, 0, 0]
    d['c2wT'] = np.ascontiguousarray(
        w2.reshape(4, 128, 6, 128).transpose(2, 0, 3, 1)).astype(bf16)
    # the patch mean's /16 is folded into the key projection
    kw = np.asarray(inp['key_w'], np.float32) * 0.0625
    d['keywT'] = np.ascontiguousarray(
        kw.reshape(2, 128, 4, 128).transpose(2, 0, 3, 1))
    w1 = np.asarray(inp['ca_w1'], np.float32)
    d['caw1T'] = np.ascontiguousarray(
        w1.reshape(64, 2, 128).transpose(1, 2, 0)).astype(bf16)
    w2c = np.asarray(inp['ca_w2'], np.float32)
    d['caw2T'] = np.ascontiguousarray(
        w2c.reshape(2, 128, 64).transpose(2, 0, 1)).astype(bf16)
    d['zeros128'] = np.zeros((128, 128), np.float32)
    for nm, key in [('qs', 'q_s'), ('qb', 'q_b'), ('c3s', 'c3_s'), ('c3b', 'c3_b'),
                    ('sa1s', 'sa1_s'), ('sa1b', 'sa1_b'), ('sa2s', 'sa2_s'),
                    ('sa2b', 'sa2_b'), ('c2s', 'conv2_s'), ('c2b', 'conv2_b'),
                    ('cab1', 'ca_b1'), ('cab2', 'ca_b2')]:
        d[nm] = np.ascontiguousarray(np.asarray(inp[key], np.float32))
    return d


def build_nc(B=4, debug_taps=()):
    nc = bacc.Bacc(None)
    x1 = nc.dram_tensor("x1", [B, 512, 20, 20], F32, kind="ExternalInput")
    x2 = nc.dram_tensor("x2", [B, 512, 40, 40], F32, kind="ExternalInput")
    w = {}
    w['qwT'] = nc.dram_tensor("qwT", [4, 2, 128, 9 * 128], BF16, kind="ExternalInput")
    w['c3w8'] = nc.dram_tensor("c3w8", [2, 4, 128, 9 * 2 * 128], F8E4, kind="ExternalInput")
    w['sa1wT'] = nc.dram_tensor("sa1wT", [2, 2, 128, 9 * 128], BF16, kind="ExternalInput")
    w['sa2wT'] = nc.dram_tensor("sa2wT", [2, 2, 128, 9 * 128], BF16, kind="ExternalInput")
    w['c2wT'] = nc.dram_tensor("c2wT", [6, 4, 128, 128], BF16, kind="ExternalInput")
    w['keywT'] = nc.dram_tensor("keywT", [4, 2, 128, 128], F32R, kind="ExternalInput")
    w['caw1T'] = nc.dram_tensor("caw1T", [2, 128, 64], BF16, kind="ExternalInput")
    w['caw2T'] = nc.dram_tensor("caw2T", [64, 2, 128], BF16, kind="ExternalInput")
    for nm, n in [('qs', 256), ('qb', 256), ('c3s', 512), ('c3b', 512),
                  ('sa1s', 256), ('sa1b', 256), ('sa2s', 256), ('sa2b', 256),
                  ('c2s', 512), ('c2b', 512), ('cab1', 64), ('cab2', 256)]:
        w[nm] = nc.dram_tensor(nm, [n], F32, kind="ExternalInput")
    w['zeros128'] = nc.dram_tensor("zeros128", [128, 128], F32R, kind="ExternalInput")
    out = nc.dram_tensor("out", [B, 1024, 40, 40], F32, kind="ExternalOutput")

    taps = {}
    if 'q' in debug_taps:
        taps['q'] = nc.dram_tensor("tap_q", [2, 128, B * 100], F32, kind="ExternalOutput")
    if 'k' in debug_taps:
        taps['k'] = nc.dram_tensor("tap_k", [2, 128, B * 100], F32, kind="ExternalOutput")
    if 'attn' in debug_taps:
        taps['attn'] = nc.dram_tensor("tap_attn", [B, 4, 128, PADPIX], F32, kind="ExternalOutput")
    if 'gate' in debug_taps:
        taps['gate'] = nc.dram_tensor("tap_gate", [B, 2, 128, 1600], F32, kind="ExternalOutput")
    if 'a2' in debug_taps:
        taps['a2'] = nc.dram_tensor("tap_a2", [B, 2, 128, 1600], F32, kind="ExternalOutput")

    with TileContext(nc) as tc:
        _emit(nc, tc, B, x1, x2, w, out, taps)
    nc.finalize()
    return nc


def _apron_memset(nc, t):
    nc.gpsimd.memset(t[:, 0:WP], 0.0)
    nc.gpsimd.memset(t[:, 41 * WP:42 * WP], 0.0)
    g = t[:].rearrange("p (y x) -> p y x", x=WP)
    nc.gpsimd.memset(g[:, 1:41, 0:1], 0.0)
    nc.gpsimd.memset(g[:, 1:41, 41:42], 0.0)


def _apron_zero_dma(nc, t, zdram):
    # f32r tiles can't be memset; DMA zeros from DRAM (same dtype, no cast)
    g = t[:].rearrange("p (y x) -> p y x", x=WP)
    rows = g[:, 0:42:41, :]            # rows 0 and 41
    nc.sync.dma_start(out=rows, in_=zdram.ap()[:, 0:84].rearrange("p (a b) -> p a b", a=2))
    nc.sync.dma_start(out=g[:, 1:41, 0:1], in_=zdram.ap()[:, 0:40])
    nc.sync.dma_start(out=g[:, 1:41, 41:42], in_=zdram.ap()[:, 0:40])


def _emit(nc, tc, B, x1, x2, w, out, taps):
    import contextlib
    ctx = contextlib.ExitStack()
    with ctx:
        from concourse import masks
        mp = ctx.enter_context(tc.tile_pool(name="main", bufs=1))
        psC = ctx.enter_context(tc.tile_pool(name="psC", bufs=2, space="PSUM"))
        psY = ctx.enter_context(tc.tile_pool(name="psY", bufs=4, space="PSUM"))
        psT = ctx.enter_context(tc.tile_pool(name="psT", bufs=2, space="PSUM"))

        ones_bf = mp.tile([128, 1], BF16, tag="ones")
        nc.gpsimd.memset(ones_bf[:], 1.0)
        ident = mp.tile([128, 128], BF16, tag="ident")
        masks.make_identity(nc, ident[:])

        def load_vec(name, n):
            p = min(n, 128)
            t = mp.tile([128, max(n // 128, 1)], F32, tag=f"vec_{name}")
            nc.sync.dma_start(out=t[0:p, 0:max(n // 128, 1)],
                              in_=w[name].ap().rearrange("(a p) -> p a", p=p))
            return t
        vs = {nm: load_vec(nm, n) for nm, n in
              [('qs', 256), ('qb', 256), ('c3s', 512), ('c3b', 512),
               ('sa1s', 256), ('sa1b', 256), ('sa2s', 256), ('sa2b', 256),
               ('c2s', 512), ('c2b', 512), ('cab1', 64), ('cab2', 256)]}

        # resident small weights
        c2_sb = mp.tile([128, 6 * 512], BF16, tag="c2w")
        for kt in range(6):
            nc.sync.dma_start(
                out=c2_sb[:, kt * 512:(kt + 1) * 512].rearrange("p (m c) -> p m c", m=4),
                in_=w['c2wT'][kt].rearrange("m p c -> p m c"))
        keyw_sb = mp.tile([128, 8 * 128], F32R, tag="keyw")
        for kt in range(4):
            nc.sync.dma_start(
                out=keyw_sb[:, kt * 256:(kt + 1) * 256].rearrange("p (m c) -> p m c", m=2),
                in_=w['keywT'][kt].rearrange("m p c -> p m c"))
        caw1_sb = mp.tile([128, 128], BF16, tag="caw1")
        for kt in range(2):
            nc.sync.dma_start(out=caw1_sb[:, kt * 64:(kt + 1) * 64], in_=w['caw1T'][kt])
        caw2_sb = mp.tile([64, 256], BF16, tag="caw2")
        nc.sync.dma_start(out=caw2_sb[:], in_=w['caw2T'].rearrange("p m c -> p (m c)"))
        # resident fp8 conv3 weights: 8 slabs of [128, 9*2*128] (g*4+mt)
        c3w8_sb = mp.tile([128, 8 * 2304], F8E4, tag="c3w8")
        for g in range(2):
            for mt in range(4):
                nc.sync.dma_start(
                    out=c3w8_sb[:, (g * 4 + mt) * 2304:(g * 4 + mt + 1) * 2304],
                    in_=w['c3w8'][g, mt])

        # x1: load + zero-padded bf16 [128, (s, 22, 22)]
        x1pad = []
        for ct in range(4):
            t = mp.tile([128, B * 484], BF16, tag=f"x1pad{ct}")
            nc.gpsimd.memset(t[:], 0.0)
            raw = mp.tile([128, B * 400], F32, tag="x2", bufs=3)
            src = x1.ap()[:, ct * 128:(ct + 1) * 128].rearrange("s p y x -> p s (y x)")
            nc.sync.dma_start(out=raw[:].rearrange("p (s a) -> p s a", s=B), in_=src)
            dst = t[:].rearrange("p (s y x) -> p s y x", s=B, x=22)[:, :, 1:21, 1:21]
            rawv = raw[:].rearrange("p (s y x) -> p s y x", s=B, x=20)
            hb = B // 2
            nc.gpsimd.tensor_copy(dst[:, 0:hb], rawv[:, 0:hb])
            nc.vector.tensor_copy(dst[:, hb:B], rawv[:, hb:B])
            x1pad.append(t)

        def up_ap(ct, s, chunk):
            y0h = chunk * CHUNK_ROWS // 2
            base = x1pad[ct][:].rearrange("p (ss a) -> p ss a", ss=B)[:, s]
            base = base.rearrange("p (y x) -> p y x", x=22)[:, 1:21, 1:21]
            up = base.unsqueeze(3).broadcast_to([128, 20, 20, 2])
            return up[:, y0h:y0h + 5]

        # ---------- q conv ----------
        qslabs = {}
        for mt in range(2):
            for kt in range(4):
                qbf = mp.tile([128, 1152], BF16, tag="wsbf", bufs=5)
                nc.sync.dma_start(out=qbf[:], in_=w['qwT'][kt, mt])
                qslabs[(kt, mt)] = qbf
        q_sb = mp.tile([128, 2 * B * 100], BF16, tag="qsb")
        for mt in range(2):
            ps0 = psC.tile([128, 512], F32, tag="cps", name="cps")
            ps = ps0[:, 0:B * 100]
            first = True
            for kt in range(4):
                base = x1pad[kt][:].rearrange("p (s y x) -> p s y x", s=B, x=22)
                for dy in range(3):
                    for dx in range(3):
                        rhs = base[:, :, dy:dy + 20:2, dx:dx + 20:2]
                        nc.tensor.matmul(
                            ps, qslabs[(kt, mt)][:, (dy * 3 + dx) * 128:(dy * 3 + dx + 1) * 128],
                            rhs, start=first, stop=(kt == 3 and dy == 2 and dx == 2))
                        first = False
            nc.scalar.activation(q_sb[:, mt * B * 100:(mt + 1) * B * 100], ps, AF.Silu,
                                 bias=vs['qb'][:, mt:mt + 1], scale=vs['qs'][:, mt:mt + 1])
        if 'q' in taps:
            for mt in range(2):
                qf = mp.tile([128, B * 100], F32, tag="tapq", bufs=1)
                nc.vector.tensor_copy(qf[:], q_sb[:, mt * B * 100:(mt + 1) * B * 100])
                nc.sync.dma_start(out=taps['q'][mt], in_=qf[:])

        k_sb = mp.tile([128, 2 * B * 100], BF16, tag="ksb")
        SCALE = 1.0 / math.sqrt(32)

        # ---------- per-sample pipeline ----------
        # persistent big tensors: aprons/garbage zeroed once, interiors
        # rewritten per sample (pool-slot rotation would re-zero every pass
        # and flood the DMA queues with tiny descriptors)
        # attn (bf16, no apron — residual only) is allocated per-sample with
        # 2 rotating slots for cross-sample overlap
        # fp8 copy of attn for conv3 DoubleRow rhs (aprons + pads zeroed once);
        # two slots so sample s+1's cast overlaps sample s's conv3 reads
        attn8 = []
        for sl in range(2):
            a8 = mp.tile([128, 4 * CTB], F8E4, tag=f"attn8_{sl}", bufs=1)
            nc.gpsimd.memset(a8[:], 0.0)
            attn8.append(a8)
        xca_slots = []
        xsa1 = []
        a2 = []
        for i in range(2):
            for sl in range(2):
                t = mp.tile([128, PADPIX], BF16, tag=f"xca{i}_{sl}", name="t", bufs=1)
                _apron_memset(nc, t)
                xca_slots.append(t)
            t2 = mp.tile([128, PADPIX], BF16, tag=f"xsa{i}", name="t2", bufs=1)
            _apron_memset(nc, t2)
            xsa1.append(t2)
            a2t = mp.tile([128, 1600], BF16, tag=f"a2_{i}", name="a2t", bufs=1)
            a2.append(a2t)

        for s in range(B):
            attn8v = attn8[s % 2][:].rearrange("p (c n) -> p c n", c=4)
            attn = [mp.tile([128, 1600], BF16, tag=f"attn{ct}", name="at", bufs=2)
                    for ct in range(4)]
            xca = [xca_slots[0 * 2 + s % 2], xca_slots[1 * 2 + s % 2]]
            # ---- MGFAB: CA gate, phase-split to batch relu/sigmoid tables ----
            hsbs = []
            for chunk in range(NCHUNK):
                # hps layout: col = r*200 + y*40 + x  (pixel (2y+r)*40+x)
                hps = psC.tile([64, 512], F32, tag="cps", name="cps")
                for r in range(2):
                    for i in range(2):
                        nc.tensor.matmul(hps[:, r * 200:(r + 1) * 200],
                                         caw1_sb[:, i * 64:(i + 1) * 64],
                                         up_ap(2 + i, s, chunk),
                                         start=(i == 0 and r == 0),
                                         stop=(i == 1 and r == 1))
                hsb = mp.tile([64, CHUNK_N], BF16, tag="hsb", bufs=4)
                hview = hsb[:].rearrange("p (y r x) -> p y r x", y=5, r=2)
                pview = hps[:, 0:400].rearrange("p (r y x) -> p y r x", r=2, y=5)
                nc.scalar.activation(hview, pview, AF.Relu, bias=vs['cab1'][0:64, 0:1])
                hsbs.append(hsb)
            for chunk in range(NCHUNK):
                gt = mp.tile([128, 2 * CHUNK_N], BF16, tag="gate", bufs=3)
                for mt in range(2):
                    gps = psC.tile([128, 512], F32, tag="cps", name="cps")
                    nc.tensor.matmul(gps[:, 0:400], caw2_sb[0:64, mt * 128:(mt + 1) * 128],
                                     hsbs[chunk][:])
                    nc.scalar.activation(gt[:, mt * CHUNK_N:(mt + 1) * CHUNK_N], gps[:, 0:400],
                                         AF.Sigmoid, bias=vs['cab2'][:, mt:mt + 1])
                y0 = chunk * CHUNK_ROWS
                for i in range(2):
                    for r in range(2):
                        dst = xca[i][:].rearrange("p (y x) -> p y x", x=WP)
                        dst = dst[:, 1 + y0 + r:1 + y0 + 10:2, 1:41]
                        g = gt[:, i * CHUNK_N:(i + 1) * CHUNK_N]
                        g = g.rearrange("p (y x) -> p y x", x=40)[:, r::2]
                        nc.gpsimd.tensor_tensor(dst, up_ap(2 + i, s, chunk), g, AluOpType.mult)
            # ---- x2 pipeline: load / patch-sum / PE-transpose value slabs ----
            kp = []
            xts = []
            for ct in range(4):
                xt = mp.tile([128, 1600], F32, tag="x2", bufs=3)
                xts.append(xt)
                nc.sync.dma_start(
                    out=xt[:], in_=x2.ap()[s, ct * 128:(ct + 1) * 128].rearrange("p y x -> p (y x)"))
                vx = xt[:].rearrange("p (phy py pwx px) -> p phy pwx py px",
                                     phy=10, py=4, pwx=10, px=4)
                kpt = mp.tile([128, 100], F32R, tag="kp", bufs=5)
                with nc.allow_low_precision(reason="f32r is fp32-width"):
                    nc.vector.tensor_reduce(kpt[:].rearrange("p (a b) -> p a b", b=10),
                                            vx, mybir.AxisListType.XY, AluOpType.add)
                kp.append(kpt)
            # regroup xt -> patch-major bf16 slabs (vector/scalar split),
            # then PE-transpose the contiguous [128,100] slabs; 4 transposes
            # pack one PSUM bank, one copy moves them to vt
            def emit_regroup(ct, engine):
                xbf = mp.tile([128, 16 * 100], BF16, tag="x2bf", bufs=4)
                xbv = xbf[:].rearrange("p (py px c) -> p py px c", py=4, px=4)
                for py in range(4):
                    srcap = xts[ct][:].rearrange("p (phy py pwx px) -> p py phy pwx px",
                                                 phy=10, py=4, pwx=10, px=4)[:, py]
                    dstap = xbv[:, py].rearrange("p px (phy pwx) -> p phy pwx px", phy=10)
                    if engine == 'v':
                        nc.vector.tensor_copy(dstap, srcap)
                    else:
                        nc.scalar.copy(dstap, srcap)
                return xbf

            def emit_transpose(xbf):
                vt = mp.tile([128, 16 * 128], BF16, tag="v", bufs=5)
                for q4 in range(4):
                    pst = psT.tile([128, 512], BF16, tag="tp", name="tp")
                    for j in range(4):
                        pp = q4 * 4 + j
                        nc.tensor.transpose(pst[0:100, j * 128:(j + 1) * 128],
                                            xbf[:, pp * 100:(pp + 1) * 100], ident[:])
                    dst = vt[0:100, q4 * 512:(q4 + 1) * 512]
                    if q4 % 2 == 0:
                        nc.scalar.activation(dst, pst[0:100, :], AF.Copy)
                    else:
                        nc.vector.tensor_copy(dst, pst[0:100, :])
                return vt

            v_ct = []
            for ct in range(2):
                v_ct.append(emit_transpose(emit_regroup(ct, 'v')))

            # ---- k projection + scores (batched) + exp + 1/sum ----
            for mt in range(2):
                psk0 = psY.tile([128, 512], F32, tag="aps", name="aps")
                psk = psk0[:, 0:100]
                for kt in range(4):
                    nc.tensor.matmul(
                        psk, keyw_sb[:, (kt * 2 + mt) * 128:(kt * 2 + mt + 1) * 128],
                        kp[kt][:], start=(kt == 0), stop=(kt == 3))
                nc.scalar.copy(k_sb[:, (mt * B + s) * 100:(mt * B + s + 1) * 100],
                               psk)
            exps = []
            for h in range(8):
                emb_ct, emb_off = h // 4, (h % 4) * 32
                pssc0 = psY.tile([128, 512], F32, tag="aps", name="aps")
                pssc = pssc0[0:100, 0:100]
                lhs = k_sb[emb_off:emb_off + 32, (emb_ct * B + s) * 100:(emb_ct * B + s + 1) * 100]
                rhs = q_sb[emb_off:emb_off + 32, (emb_ct * B + s) * 100:(emb_ct * B + s + 1) * 100]
                nc.tensor.matmul(pssc, lhs, rhs, tile_position=(emb_off, 0))
                expT = mp.tile([100, 100], BF16, tag="expT", bufs=8)
                nc.scalar.activation(expT[:], pssc, AF.Exp, scale=SCALE)
                exps.append(expT)
            recs = []
            rbcs = []
            for half in range(2):
                pssum = psY.tile([128, 512], F32, tag="aps", name="aps")
                for j in range(4):
                    nc.tensor.matmul(pssum[0:1, j * 100:(j + 1) * 100],
                                     ones_bf[0:100, 0:1], exps[half * 4 + j][:],
                                     start=(j == 0), stop=(j == 3))
                recip = mp.tile([1, 400], F32, tag="recip", bufs=3)
                nc.vector.reciprocal(recip[:], pssum[0:1, 0:400])
                rbc = mp.tile([128, 400], F32, tag="rbc", bufs=2)
                nc.gpsimd.partition_broadcast(rbc[:], recip[:])
                rbcs.append(rbc)
            for h in range(8):
                recs.append((exps[h],
                             rbcs[h // 4][:, (h % 4) * 100:(h % 4) * 100 + 100]))

            v_ct.append(emit_transpose(emit_regroup(2, 's')))
            v_ct.append(emit_transpose(emit_regroup(3, 'v')))


            if 'gate' in taps:
                for i in range(2):
                    gf = mp.tile([128, 1600], F32, tag="tapg", bufs=1)
                    for chunk in range(NCHUNK):
                        nc.vector.tensor_copy(gf[:, chunk * 400:(chunk + 1) * 400],
                                              gates[chunk][:, i * CHUNK_N:(i + 1) * CHUNK_N])
                    nc.sync.dma_start(out=taps['gate'][s, i], in_=gf[:])
            # ---- SA conv1 ----
            def stream_sa(wt):
                slabs = {}
                for mt in range(2):
                    for kt in range(2):
                        tl = mp.tile([128, 1152], BF16, tag="wsbf", bufs=5)
                        nc.sync.dma_start(out=tl[:], in_=wt[kt, mt])
                        slabs[(kt, mt)] = tl
                return slabs

            def conv3x3(src_tiles, slabs, mt, kt_n, chunk):
                ps0 = psC.tile([128, 512], F32, tag="cps", name="cps")
                ps = ps0[:, 0:CHUNK_N]
                y0 = chunk * CHUNK_ROWS
                first = True
                for kt in range(kt_n):
                    base = src_tiles[kt][:].rearrange("p (y x) -> p y x", x=WP)
                    for dy in range(3):
                        for dx in range(3):
                            rhs = base[:, y0 + dy:y0 + dy + CHUNK_ROWS, dx:dx + 40]
                            lhsT = slabs[(kt, mt)][:, (dy * 3 + dx) * 128:(dy * 3 + dx + 1) * 128]
                            nc.tensor.matmul(ps, lhsT, rhs, start=first,
                                             stop=(kt == kt_n - 1 and dy == 2 and dx == 2))
                            first = False
                return ps

            # ---- attention apply interleaved with SA conv1 (keeps PE fed
            # while vector drains assembles); per-ct fp8 cast as soon as both
            # heads of a ct block are assembled ----
            sa1slabs = stream_sa(w['sa1wT'])
            sa1_jobs = [(mt, chunk) for mt in range(2) for chunk in range(NCHUNK)]
            attn_nop = [ct_t[:].rearrange("p (y x) -> p y x", x=40) for ct_t in attn]
            for h in range(8):
                expT, rbc = recs[h]
                o = (h % 2) * 64
                for py in range(4):
                    psy = psY.tile([128, 512], F32, tag="aps", name="aps")
                    for px in range(4):
                        pp = py * 4 + px
                        lhsT = v_ct[h // 2][0:100, pp * 128 + o: pp * 128 + o + 64]
                        nc.tensor.matmul(psy[o:o + 64, px * 100:px * 100 + 100], lhsT, expT[:],
                                         start=(px == 0), stop=(px == 3))
                    dstg = attn_nop[h // 2][o:o + 64, py:40:4, :]
                    dstg = dstg.rearrange("p a (pwx px) -> p a pwx px", px=4)
                    in0 = psy[o:o + 64, 0:400].rearrange("p (px phy pwx) -> p phy pwx px",
                                                         px=4, phy=10)
                    in1 = rbc[o:o + 64].rearrange("p (a b) -> p a b", b=10)
                    in1 = in1.unsqueeze(3).broadcast_to([64, 10, 10, 4])
                    nc.vector.scalar_tensor_tensor(dstg, in0, 0.0, in1,
                                                   AluOpType.bypass, AluOpType.mult)
                if h % 2 == 1:
                    ct = h // 2
                    dst8 = attn8v[:, ct, 43:43 + PADPIX].rearrange("p (y x) -> p y x", x=42)
                    nc.vector.tensor_copy(dst8[:, 1:41, 1:41], attn_nop[ct])
                # one SA1 (mt, chunk) group between heads keeps the PE busy
                mt, chunk = sa1_jobs[h]
                ps = conv3x3(xca, sa1slabs, mt, 2, chunk)[:, 0:CHUNK_N]
                y0 = chunk * CHUNK_ROWS
                dst = xsa1[mt][:].rearrange("p (y x) -> p y x", x=WP)[:, 1 + y0:11 + y0, 1:41]
                nc.scalar.activation(dst, ps.rearrange("p (a b) -> p a b", b=40), AF.Silu,
                                     bias=vs['sa1b'][:, mt:mt + 1], scale=vs['sa1s'][:, mt:mt + 1])

            # ---- SA conv2 + residual ----
            sa2slabs = stream_sa(w['sa2wT'])
            for mt in range(2):
                for chunk in range(NCHUNK):
                    ps = conv3x3(xsa1, sa2slabs, mt, 2, chunk)[:, 0:CHUNK_N]
                    y0 = chunk * CHUNK_ROWS
                    tsilu = mp.tile([128, CHUNK_N], F32, tag="silu", bufs=3)
                    nc.scalar.activation(tsilu[:], ps, AF.Silu,
                                         bias=vs['sa2b'][:, mt:mt + 1], scale=vs['sa2s'][:, mt:mt + 1])
                    xc = xca[mt][:].rearrange("p (y x) -> p y x", x=WP)[:, 1 + y0:11 + y0, 1:41]
                    nc.gpsimd.tensor_tensor(a2[mt][:, y0 * 40:(y0 + 10) * 40],
                                            tsilu[:].rearrange("p (a b) -> p a b", b=40),
                                            xc, AluOpType.add)
            if 'a2' in taps:
                for i in range(2):
                    af = mp.tile([128, 1600], F32, tag="tapg", bufs=1)
                    nc.vector.tensor_copy(af[:], a2[i][:])
                    nc.sync.dma_start(out=taps['a2'][s, i], in_=af[:])

            # ---- conv3 (fp8 DoubleRow) + residual -> x2_out ----
            for mt in range(4):
                for chunk in range(NCHUNK):
                    ps0 = psC.tile([128, 512], F32, tag="cps", name="cps")
                    ps = ps0[:, 0:420]
                    y0 = chunk * CHUNK_ROWS
                    first = True
                    for g in range(2):
                        wslab = c3w8_sb[:, (g * 4 + mt) * 2304:(g * 4 + mt + 1) * 2304] \
                            .rearrange("p (k two m) -> p k two m", two=2, m=128)
                        for dy in range(3):
                            for dx in range(3):
                                s0 = 42 + (y0 + dy) * 42 + dx
                                rhs = attn8v[:, 2 * g:2 * g + 2, s0:s0 + 420]
                                nc.tensor.matmul(ps, wslab[:, dy * 3 + dx], rhs,
                                                 start=first,
                                                 stop=(g == 1 and dy == 2 and dx == 2),
                                                 perf_mode=mybir.MatmulPerfMode.DoubleRow)
                                first = False
                    tsilu = mp.tile([128, CHUNK_N], F32, tag="silu", bufs=3)
                    psv = ps.rearrange("p (y x) -> p y x", x=42)[:, :, 1:41]
                    nc.scalar.activation(tsilu[:].rearrange("p (a b) -> p a b", b=40),
                                         psv, AF.Silu,
                                         bias=vs['c3b'][:, mt:mt + 1], scale=vs['c3s'][:, mt:mt + 1])
                    osb = mp.tile([128, CHUNK_N], F32, tag="osb", bufs=4)
                    at2 = attn_nop[mt][:, y0:y0 + 10, :]
                    nc.gpsimd.tensor_tensor(osb[:].rearrange("p (a b) -> p a b", b=40),
                                            tsilu[:].rearrange("p (a b) -> p a b", b=40),
                                            at2, AluOpType.add)
                    nc.sync.dma_start(
                        out=out.ap()[s, 512 + mt * 128:512 + (mt + 1) * 128]
                            .rearrange("p y x -> p (y x)")[:, y0 * 40:(y0 + 10) * 40],
                        in_=osb[:])

            # ---- conv2 (1x1) -> x1_out ----
            for mt in range(4):
                for chunk in range(NCHUNK):
                    # ps layout: col = r*200 + y*40 + x  (pixel (2y+r)*40+x)
                    ps0 = psC.tile([128, 512], F32, tag="cps", name="cps")
                    ps = ps0[:, 0:CHUNK_N]
                    y0 = chunk * CHUNK_ROWS
                    first = True
                    for r in range(2):
                        for kt in range(4):
                            nc.tensor.matmul(
                                ps[:, r * 200:(r + 1) * 200],
                                c2_sb[:, (kt * 4 + mt) * 128:(kt * 4 + mt + 1) * 128],
                                up_ap(kt, s, chunk), start=first, stop=False)
                            first = False
                    for i in range(2):
                        kt = 4 + i
                        rhs = a2[i][:, y0 * 40:(y0 + 10) * 40]
                        rhs = rhs.rearrange("p (y r x) -> p r y x", y=5, r=2)
                        nc.tensor.matmul(ps, c2_sb[:, (kt * 4 + mt) * 128:(kt * 4 + mt + 1) * 128],
                                         rhs, start=False, stop=(i == 1))
                    osb = mp.tile([128, CHUNK_N], F32, tag="osb", bufs=4)
                    oview = osb[:].rearrange("p (y r x) -> p y r x", y=5, r=2)
                    pv = ps.rearrange("p (r y x) -> p y r x", r=2, y=5)
                    nc.scalar.activation(oview, pv, AF.Silu,
                                         bias=vs['c2b'][:, mt:mt + 1], scale=vs['c2s'][:, mt:mt + 1])
                    nc.sync.dma_start(
                        out=out.ap()[s, mt * 128:(mt + 1) * 128]
                            .rearrange("p y x -> p (y x)")[:, y0 * 40:(y0 + 10) * 40],
                        in_=osb[:])



# revision 46
# speedup vs baseline: 1.0143x; 1.0103x over previous
"""Trainium2 Bass kernel for nn_CSFAProV2 — full-input contract.

kernel(**inputs) takes the FULL unsharded inputs (B=32), shards the batch
across 8 NeuronCores (4 samples each, pure data parallel over axis 0 of
x1/x2, weights replicated), compiles+runs the Bass/Tile kernel via
run_bass_kernel_spmd, and concatenates the per-core outputs into the full
[32, 1024, 40, 40] result. Self-contained: the Bass kernel builder is
inlined below; only needs /opt/trn_rl_repo (concourse) + numpy/ml_dtypes.
"""

import sys

if '/opt/trn_rl_repo' not in sys.path:
    sys.path.insert(0, '/opt/trn_rl_repo')

import numpy as np

N_CORES = 8
B_FULL = 32
B_CORE = B_FULL // N_CORES

_cache = {}


def make_in_maps(inputs):
    wd = prep_weights(inputs)
    x1 = np.ascontiguousarray(np.asarray(inputs['x1'], np.float32))
    x2 = np.ascontiguousarray(np.asarray(inputs['x2'], np.float32))

    in_maps = []
    for c in range(N_CORES):
        m = dict(wd)
        m['x1'] = x1[c * B_CORE:(c + 1) * B_CORE]
        m['x2'] = x2[c * B_CORE:(c + 1) * B_CORE]
        in_maps.append(m)
    return in_maps


def kernel(**inputs):
    from concourse.bass_utils import run_bass_kernel_spmd

    if 'nc' not in _cache:
        _cache['nc'] = build_nc(B=B_CORE)
    nc = _cache['nc']

    in_maps = make_in_maps(inputs)
    res = run_bass_kernel_spmd(nc, in_maps, core_ids=list(range(N_CORES)))
    return np.concatenate([res.results[c]['out'] for c in range(N_CORES)], axis=0)


# ======================================================================
# Inlined Bass/Tile kernel builder (generated from kernel_lib.py)
# ======================================================================

"""Bass/Tile kernel builder for nn_CSFAProV2 (per-core, B batch samples).

q-conv (stride-2 3x3, bf16) -> cross-attention (transposed softmax;
DMA-transposed patch-major bf16 value slabs) -> conv3 3x3 fp32r + residual;
MGFAB branch (channel-MLP sigmoid gate, two 3x3 convs, 1x1 conv) in bf16.
Channel-on-partition layouts; convs = PE matmuls accumulating over
(ktile, dy, dx) on padded-spatial SBUF tiles.
"""

import math
import numpy as np
import concourse.bass as bass
import concourse.mybir as mybir
from concourse import bacc
from concourse.tile import TileContext
from concourse.alu_op_type import AluOpType

F32 = mybir.dt.float32
F32R = mybir.dt.float32r
BF16 = mybir.dt.bfloat16
F8E4 = mybir.dt.float8e4
AF = mybir.ActivationFunctionType

H = W = 40
WP = 42
PADPIX = WP * WP
CTB = PADPIX + 2 * 43  # fp8 attn block: 43 pad | 1764 | 43 pad
CHUNK_ROWS = 10
NCHUNK = H // CHUNK_ROWS
CHUNK_N = CHUNK_ROWS * W
VCOL = 128


def prep_weights(inp):
    import ml_dtypes
    bf16 = ml_dtypes.bfloat16
    d = {}

    def convT(w, kt_n, mt_n):  # [Cout, Cin, 3, 3] -> [kt, mt, 128, 9*128]
        x = np.asarray(w, np.float32).reshape(mt_n, 128, kt_n, 128, 9)
        x = x.transpose(2, 0, 3, 4, 1)
        return np.ascontiguousarray(x.reshape(kt_n, mt_n, 128, 9 * 128))

    d['qwT'] = convT(inp['q_w'], 4, 2).astype(bf16)
    # conv3 weights for fp8 DoubleRow: [pair g, mt, ki, (dydx, j, mo)]
    c3 = np.asarray(inp['c3_w'], np.float32).reshape(4, 128, 2, 2, 128, 9)
    c3 = c3.transpose(2, 0, 4, 5, 3, 1)  # g, mt, ki, dydx, j, mo
    d['c3w8'] = np.ascontiguousarray(c3.reshape(2, 4, 128, 9 * 2 * 128)).astype(
        ml_dtypes.float8_e4m3)
    d['sa1wT'] = convT(inp['sa1_w'], 2, 2).astype(bf16)
    d['sa2wT'] = convT(inp['sa2_w'], 2, 2).astype(bf16)
    w2 = np.asarray(inp['conv2_w'], np.float32)[:, :, 0, 0]
    d['c2wT'] = np.ascontiguousarray(
        w2.reshape(4, 128, 6, 128).transpose(2, 0, 3, 1)).astype(bf16)
    # the patch mean's /16 is folded into the key projection
    kw = np.asarray(inp['key_w'], np.float32) * 0.0625
    d['keywT'] = np.ascontiguousarray(
        kw.reshape(2, 128, 4, 128).transpose(2, 0, 3, 1))
    w1 = np.asarray(inp['ca_w1'], np.float32)
    d['caw1T'] = np.ascontiguousarray(
        w1.reshape(64, 2, 128).transpose(1, 2, 0)).astype(bf16)
    w2c = np.asarray(inp['ca_w2'], np.float32)
    d['caw2T'] = np.ascontiguousarray(
        w2c.reshape(2, 128, 64).transpose(2, 0, 1)).astype(bf16)
    d['zeros128'] = np.zeros((128, 128), np.float32)
    for nm, key in [('qs', 'q_s'), ('qb', 'q_b'), ('c3s', 'c3_s'), ('c3b', 'c3_b'),
                    ('sa1s', 'sa1_s'), ('sa1b', 'sa1_b'), ('sa2s', 'sa2_s'),
                    ('sa2b', 'sa2_b'), ('c2s', 'conv2_s'), ('c2b', 'conv2_b'),
                    ('cab1', 'ca_b1'), ('cab2', 'ca_b2')]:
        d[nm] = np.ascontiguousarray(np.asarray(inp[key], np.float32))
    return d


def build_nc(B=4, debug_taps=()):
    nc = bacc.Bacc(None)
    x1 = nc.dram_tensor("x1", [B, 512, 20, 20], F32, kind="ExternalInput")
    x2 = nc.dram_tensor("x2", [B, 512, 40, 40], F32, kind="ExternalInput")
    w = {}
    w['qwT'] = nc.dram_tensor("qwT", [4, 2, 128, 9 * 128], BF16, kind="ExternalInput")
    w['c3w8'] = nc.dram_tensor("c3w8", [2, 4, 128, 9 * 2 * 128], F8E4, kind="ExternalInput")
    w['sa1wT'] = nc.dram_tensor("sa1wT", [2, 2, 128, 9 * 128], BF16, kind="ExternalInput")
    w['sa2wT'] = nc.dram_tensor("sa2wT", [2, 2, 128, 9 * 128], BF16, kind="ExternalInput")
    w['c2wT'] = nc.dram_tensor("c2wT", [6, 4, 128, 128], BF16, kind="ExternalInput")
    w['keywT'] = nc.dram_tensor("keywT", [4, 2, 128, 128], F32R, kind="ExternalInput")
    w['caw1T'] = nc.dram_tensor("caw1T", [2, 128, 64], BF16, kind="ExternalInput")
    w['caw2T'] = nc.dram_tensor("caw2T", [64, 2, 128], BF16, kind="ExternalInput")
    for nm, n in [('qs', 256), ('qb', 256), ('c3s', 512), ('c3b', 512),
                  ('sa1s', 256), ('sa1b', 256), ('sa2s', 256), ('sa2b', 256),
                  ('c2s', 512), ('c2b', 512), ('cab1', 64), ('cab2', 256)]:
        w[nm] = nc.dram_tensor(nm, [n], F32, kind="ExternalInput")
    w['zeros128'] = nc.dram_tensor("zeros128", [128, 128], F32R, kind="ExternalInput")
    out = nc.dram_tensor("out", [B, 1024, 40, 40], F32, kind="ExternalOutput")

    taps = {}
    if 'q' in debug_taps:
        taps['q'] = nc.dram_tensor("tap_q", [2, 128, B * 100], F32, kind="ExternalOutput")
    if 'k' in debug_taps:
        taps['k'] = nc.dram_tensor("tap_k", [2, 128, B * 100], F32, kind="ExternalOutput")
    if 'attn' in debug_taps:
        taps['attn'] = nc.dram_tensor("tap_attn", [B, 4, 128, PADPIX], F32, kind="ExternalOutput")
    if 'gate' in debug_taps:
        taps['gate'] = nc.dram_tensor("tap_gate", [B, 2, 128, 1600], F32, kind="ExternalOutput")
    if 'a2' in debug_taps:
        taps['a2'] = nc.dram_tensor("tap_a2", [B, 2, 128, 1600], F32, kind="ExternalOutput")

    with TileContext(nc) as tc:
        _emit(nc, tc, B, x1, x2, w, out, taps)
    nc.finalize()
    return nc


def _apron_memset(nc, t):
    nc.gpsimd.memset(t[:, 0:WP], 0.0)
    nc.gpsimd.memset(t[:, 41 * WP:42 * WP], 0.0)
    g = t[:].rearrange("p (y x) -> p y x", x=WP)
    nc.gpsimd.memset(g[:, 1:41, 0:1], 0.0)
    nc.gpsimd.memset(g[:, 1:41, 41:42], 0.0)


def _apron_zero_dma(nc, t, zdram):
    # f32r tiles can't be memset; DMA zeros from DRAM (same dtype, no cast)
    g = t[:].rearrange("p (y x) -> p y x", x=WP)
    rows = g[:, 0:42:41, :]            # rows 0 and 41
    nc.sync.dma_start(out=rows, in_=zdram.ap()[:, 0:84].rearrange("p (a b) -> p a b", a=2))
    nc.sync.dma_start(out=g[:, 1:41, 0:1], in_=zdram.ap()[:, 0:40])
    nc.sync.dma_start(out=g[:, 1:41, 41:42], in_=zdram.ap()[:, 0:40])


def _emit(nc, tc, B, x1, x2, w, out, taps):
    import contextlib
    ctx = contextlib.ExitStack()
    with ctx:
        from concourse import masks
        mp = ctx.enter_context(tc.tile_pool(name="main", bufs=1))
        psC = ctx.enter_context(tc.tile_pool(name="psC", bufs=2, space="PSUM"))
        psY = ctx.enter_context(tc.tile_pool(name="psY", bufs=4, space="PSUM"))
        psT = ctx.enter_context(tc.tile_pool(name="psT", bufs=2, space="PSUM"))

        ones_bf = mp.tile([128, 1], BF16, tag="ones")
        nc.gpsimd.memset(ones_bf[:], 1.0)
        ident = mp.tile([128, 128], BF16, tag="ident")
        masks.make_identity(nc, ident[:])

        def load_vec(name, n):
            p = min(n, 128)
            t = mp.tile([128, max(n // 128, 1)], F32, tag=f"vec_{name}")
            nc.sync.dma_start(out=t[0:p, 0:max(n // 128, 1)],
                              in_=w[name].ap().rearrange("(a p) -> p a", p=p))
            return t
        vs = {nm: load_vec(nm, n) for nm, n in
              [('qs', 256), ('qb', 256), ('c3s', 512), ('c3b', 512),
               ('sa1s', 256), ('sa1b', 256), ('sa2s', 256), ('sa2b', 256),
               ('c2s', 512), ('c2b', 512), ('cab1', 64), ('cab2', 256)]}

        # resident small weights
        c2_sb = mp.tile([128, 6 * 512], BF16, tag="c2w")
        for kt in range(6):
            nc.sync.dma_start(
                out=c2_sb[:, kt * 512:(kt + 1) * 512].rearrange("p (m c) -> p m c", m=4),
                in_=w['c2wT'][kt].rearrange("m p c -> p m c"))
        keyw_sb = mp.tile([128, 8 * 128], F32R, tag="keyw")
        for kt in range(4):
            nc.sync.dma_start(
                out=keyw_sb[:, kt * 256:(kt + 1) * 256].rearrange("p (m c) -> p m c", m=2),
                in_=w['keywT'][kt].rearrange("m p c -> p m c"))
        caw1_sb = mp.tile([128, 128], BF16, tag="caw1")
        for kt in range(2):
            nc.sync.dma_start(out=caw1_sb[:, kt * 64:(kt + 1) * 64], in_=w['caw1T'][kt])
        caw2_sb = mp.tile([64, 256], BF16, tag="caw2")
        nc.sync.dma_start(out=caw2_sb[:], in_=w['caw2T'].rearrange("p m c -> p (m c)"))
        # resident fp8 conv3 weights: 8 slabs of [128, 9*2*128] (g*4+mt)
        c3w8_sb = mp.tile([128, 8 * 2304], F8E4, tag="c3w8")
        for g in range(2):
            for mt in range(4):
                nc.sync.dma_start(
                    out=c3w8_sb[:, (g * 4 + mt) * 2304:(g * 4 + mt + 1) * 2304],
                    in_=w['c3w8'][g, mt])

        # x1: load + zero-padded bf16 [128, (s, 22, 22)]
        x1pad = []
        for ct in range(4):
            t = mp.tile([128, B * 484], BF16, tag=f"x1pad{ct}")
            nc.gpsimd.memset(t[:], 0.0)
            raw = mp.tile([128, B * 400], F32, tag="x2", bufs=3)
            src = x1.ap()[:, ct * 128:(ct + 1) * 128].rearrange("s p y x -> p s (y x)")
            nc.sync.dma_start(out=raw[:].rearrange("p (s a) -> p s a", s=B), in_=src)
            dst = t[:].rearrange("p (s y x) -> p s y x", s=B, x=22)[:, :, 1:21, 1:21]
            rawv = raw[:].rearrange("p (s y x) -> p s y x", s=B, x=20)
            hb = B // 2
            nc.gpsimd.tensor_copy(dst[:, 0:hb], rawv[:, 0:hb])
            nc.vector.tensor_copy(dst[:, hb:B], rawv[:, hb:B])
            x1pad.append(t)

        def up_ap(ct, s, chunk):
            y0h = chunk * CHUNK_ROWS // 2
            base = x1pad[ct][:].rearrange("p (ss a) -> p ss a", ss=B)[:, s]
            base = base.rearrange("p (y x) -> p y x", x=22)[:, 1:21, 1:21]
            up = base.unsqueeze(3).broadcast_to([128, 20, 20, 2])
            return up[:, y0h:y0h + 5]

        # ---------- q conv ----------
        qslabs = {}
        for mt in range(2):
            for kt in range(4):
                qbf = mp.tile([128, 1152], BF16, tag="wsbf", bufs=5)
                nc.sync.dma_start(out=qbf[:], in_=w['qwT'][kt, mt])
                qslabs[(kt, mt)] = qbf
        q_sb = mp.tile([128, 2 * B * 100], BF16, tag="qsb")
        for mt in range(2):
            ps0 = psC.tile([128, 512], F32, tag="cps", name="cps")
            ps = ps0[:, 0:B * 100]
            first = True
            for kt in range(4):
                base = x1pad[kt][:].rearrange("p (s y x) -> p s y x", s=B, x=22)
                for dy in range(3):
                    for dx in range(3):
                        rhs = base[:, :, dy:dy + 20:2, dx:dx + 20:2]
                        nc.tensor.matmul(
                            ps, qslabs[(kt, mt)][:, (dy * 3 + dx) * 128:(dy * 3 + dx + 1) * 128],
                            rhs, start=first, stop=(kt == 3 and dy == 2 and dx == 2))
                        first = False
            nc.scalar.activation(q_sb[:, mt * B * 100:(mt + 1) * B * 100], ps, AF.Silu,
                                 bias=vs['qb'][:, mt:mt + 1], scale=vs['qs'][:, mt:mt + 1])
        if 'q' in taps:
            for mt in range(2):
                qf = mp.tile([128, B * 100], F32, tag="tapq", bufs=1)
                nc.vector.tensor_copy(qf[:], q_sb[:, mt * B * 100:(mt + 1) * B * 100])
                nc.sync.dma_start(out=taps['q'][mt], in_=qf[:])

        k_sb = mp.tile([128, 2 * B * 100], BF16, tag="ksb")
        SCALE = 1.0 / math.sqrt(32)

        # ---------- per-sample pipeline ----------
        # persistent big tensors: aprons/garbage zeroed once, interiors
        # rewritten per sample (pool-slot rotation would re-zero every pass
        # and flood the DMA queues with tiny descriptors)
        # attn (bf16, no apron — residual only) is allocated per-sample with
        # 2 rotating slots for cross-sample overlap
        # fp8 copy of attn for conv3 DoubleRow rhs (aprons + pads zeroed once);
        # two slots so sample s+1's cast overlaps sample s's conv3 reads
        attn8 = []
        for sl in range(2):
            a8 = mp.tile([128, 4 * CTB], F8E4, tag=f"attn8_{sl}", bufs=1)
            nc.gpsimd.memset(a8[:], 0.0)
            attn8.append(a8)
        xca_slots = []
        xsa1 = []
        a2 = []
        for i in range(2):
            for sl in range(2):
                t = mp.tile([128, PADPIX], BF16, tag=f"xca{i}_{sl}", name="t", bufs=1)
                _apron_memset(nc, t)
                xca_slots.append(t)
            t2 = mp.tile([128, PADPIX], BF16, tag=f"xsa{i}", name="t2", bufs=1)
            _apron_memset(nc, t2)
            xsa1.append(t2)
            a2t = mp.tile([128, 1600], BF16, tag=f"a2_{i}", name="a2t", bufs=1)
            a2.append(a2t)

        for s in range(B):
            attn8v = attn8[s % 2][:].rearrange("p (c n) -> p c n", c=4)
            attn = [mp.tile([128, 1600], BF16, tag=f"attn{ct}", name="at", bufs=2)
                    for ct in range(4)]
            xca = [xca_slots[0 * 2 + s % 2], xca_slots[1 * 2 + s % 2]]
            # ---- MGFAB: CA gate, phase-split to batch relu/sigmoid tables ----
            hsbs = []
            for chunk in range(NCHUNK):
                # hps layout: col = r*200 + y*40 + x  (pixel (2y+r)*40+x)
                hps = psC.tile([64, 512], F32, tag="cps", name="cps")
                for r in range(2):
                    for i in range(2):
                        nc.tensor.matmul(hps[:, r * 200:(r + 1) * 200],
                                         caw1_sb[:, i * 64:(i + 1) * 64],
                                         up_ap(2 + i, s, chunk),
                                         start=(i == 0 and r == 0),
                                         stop=(i == 1 and r == 1))
                hsb = mp.tile([64, CHUNK_N], BF16, tag="hsb", bufs=4)
                hview = hsb[:].rearrange("p (y r x) -> p y r x", y=5, r=2)
                pview = hps[:, 0:400].rearrange("p (r y x) -> p y r x", r=2, y=5)
                nc.scalar.activation(hview, pview, AF.Relu, bias=vs['cab1'][0:64, 0:1])
                hsbs.append(hsb)
            for chunk in range(NCHUNK):
                gt = mp.tile([128, 2 * CHUNK_N], BF16, tag="gate", bufs=3)
                for mt in range(2):
                    gps = psC.tile([128, 512], F32, tag="cps", name="cps")
                    nc.tensor.matmul(gps[:, 0:400], caw2_sb[0:64, mt * 128:(mt + 1) * 128],
                                     hsbs[chunk][:])
                    nc.scalar.activation(gt[:, mt * CHUNK_N:(mt + 1) * CHUNK_N], gps[:, 0:400],
                                         AF.Sigmoid, bias=vs['cab2'][:, mt:mt + 1])
                y0 = chunk * CHUNK_ROWS
                for i in range(2):
                    for r in range(2):
                        dst = xca[i][:].rearrange("p (y x) -> p y x", x=WP)
                        dst = dst[:, 1 + y0 + r:1 + y0 + 10:2, 1:41]
                        g = gt[:, i * CHUNK_N:(i + 1) * CHUNK_N]
                        g = g.rearrange("p (y x) -> p y x", x=40)[:, r::2]
                        nc.gpsimd.tensor_tensor(dst, up_ap(2 + i, s, chunk), g, AluOpType.mult)
            # ---- x2 pipeline: load / patch-sum / PE-transpose value slabs ----
            kp = []
            xts = []
            for ct in range(4):
                xt = mp.tile([128, 1600], F32, tag="x2", bufs=3)
                xts.append(xt)
                nc.sync.dma_start(
                    out=xt[:], in_=x2.ap()[s, ct * 128:(ct + 1) * 128].rearrange("p y x -> p (y x)"))
                vx = xt[:].rearrange("p (phy py pwx px) -> p phy pwx py px",
                                     phy=10, py=4, pwx=10, px=4)
                kpt = mp.tile([128, 100], F32R, tag="kp", bufs=5)
                with nc.allow_low_precision(reason="f32r is fp32-width"):
                    nc.vector.tensor_reduce(kpt[:].rearrange("p (a b) -> p a b", b=10),
                                            vx, mybir.AxisListType.XY, AluOpType.add)
                kp.append(kpt)
            # regroup xt -> patch-major bf16 slabs (vector/scalar split),
            # then PE-transpose the contiguous [128,100] slabs; 4 transposes
            # pack one PSUM bank, one copy moves them to vt
            def emit_regroup(ct, engine):
                xbf = mp.tile([128, 16 * 100], BF16, tag="x2bf", bufs=4)
                xbv = xbf[:].rearrange("p (py px c) -> p py px c", py=4, px=4)
                for py in range(4):
                    srcap = xts[ct][:].rearrange("p (phy py pwx px) -> p py phy pwx px",
                                                 phy=10, py=4, pwx=10, px=4)[:, py]
                    dstap = xbv[:, py].rearrange("p px (phy pwx) -> p phy pwx px", phy=10)
                    if engine == 'v':
                        nc.vector.tensor_copy(dstap, srcap)
                    else:
                        nc.scalar.copy(dstap, srcap)
                return xbf

            def emit_transpose(xbf):
                vt = mp.tile([128, 16 * 128], BF16, tag="v", bufs=5)
                for q4 in range(4):
                    pst = psT.tile([128, 512], BF16, tag="tp", name="tp")
                    for j in range(4):
                        pp = q4 * 4 + j
                        nc.tensor.transpose(pst[0:100, j * 128:(j + 1) * 128],
                                            xbf[:, pp * 100:(pp + 1) * 100], ident[:])
                    dst = vt[0:100, q4 * 512:(q4 + 1) * 512]
                    if q4 % 2 == 0:
                        nc.scalar.activation(dst, pst[0:100, :], AF.Copy)
                    else:
                        nc.vector.tensor_copy(dst, pst[0:100, :])
                return vt

            v_ct = []
            for ct in range(2):
                v_ct.append(emit_transpose(emit_regroup(ct, 'v')))

            # ---- k projection + scores (batched) + exp + 1/sum ----
            for mt in range(2):
                psk0 = psY.tile([128, 512], F32, tag="aps", name="aps")
                psk = psk0[:, 0:100]
                for kt in range(4):
                    nc.tensor.matmul(
                        psk, keyw_sb[:, (kt * 2 + mt) * 128:(kt * 2 + mt + 1) * 128],
                        kp[kt][:], start=(kt == 0), stop=(kt == 3))
                nc.scalar.copy(k_sb[:, (mt * B + s) * 100:(mt * B + s + 1) * 100],
                               psk)
            exps = []
            for h in range(8):
                emb_ct, emb_off = h // 4, (h % 4) * 32
                pssc0 = psY.tile([128, 512], F32, tag="aps", name="aps")
                pssc = pssc0[0:100, 0:100]
                lhs = k_sb[emb_off:emb_off + 32, (emb_ct * B + s) * 100:(emb_ct * B + s + 1) * 100]
                rhs = q_sb[emb_off:emb_off + 32, (emb_ct * B + s) * 100:(emb_ct * B + s + 1) * 100]
                nc.tensor.matmul(pssc, lhs, rhs, tile_position=(emb_off, 0))
                expT = mp.tile([100, 100], BF16, tag="expT", bufs=8)
                nc.scalar.activation(expT[:], pssc, AF.Exp, scale=SCALE)
                exps.append(expT)
            recs = []
            rbcs = []
            for half in range(2):
                pssum = psY.tile([128, 512], F32, tag="aps", name="aps")
                for j in range(4):
                    nc.tensor.matmul(pssum[0:1, j * 100:(j + 1) * 100],
                                     ones_bf[0:100, 0:1], exps[half * 4 + j][:],
                                     start=(j == 0), stop=(j == 3))
                recip = mp.tile([1, 400], F32, tag="recip", bufs=3)
                nc.vector.reciprocal(recip[:], pssum[0:1, 0:400])
                rbc = mp.tile([128, 400], F32, tag="rbc", bufs=2)
                nc.gpsimd.partition_broadcast(rbc[:], recip[:])
                rbcs.append(rbc)
            for h in range(8):
                recs.append((exps[h],
                             rbcs[h // 4][:, (h % 4) * 100:(h % 4) * 100 + 100]))

            for ct in range(2, 4):
                v_ct.append(emit_transpose(emit_regroup(ct, 'v')))


            if 'gate' in taps:
                for i in range(2):
                    gf = mp.tile([128, 1600], F32, tag="tapg", bufs=1)
                    for chunk in range(NCHUNK):
                        nc.vector.tensor_copy(gf[:, chunk * 400:(chunk + 1) * 400],
                                              gates[chunk][:, i * CHUNK_N:(i + 1) * CHUNK_N])
                    nc.sync.dma_start(out=taps['gate'][s, i], in_=gf[:])
            # ---- SA conv1 ----
            def stream_sa(wt):
                slabs = {}
                for mt in range(2):
                    for kt in range(2):
                        tl = mp.tile([128, 1152], BF16, tag="wsbf", bufs=5)
                        nc.sync.dma_start(out=tl[:], in_=wt[kt, mt])
                        slabs[(kt, mt)] = tl
                return slabs

            def conv3x3(src_tiles, slabs, mt, kt_n, chunk):
                ps0 = psC.tile([128, 512], F32, tag="cps", name="cps")
                ps = ps0[:, 0:CHUNK_N]
                y0 = chunk * CHUNK_ROWS
                first = True
                for kt in range(kt_n):
                    base = src_tiles[kt][:].rearrange("p (y x) -> p y x", x=WP)
                    for dy in range(3):
                        for dx in range(3):
                            rhs = base[:, y0 + dy:y0 + dy + CHUNK_ROWS, dx:dx + 40]
                            lhsT = slabs[(kt, mt)][:, (dy * 3 + dx) * 128:(dy * 3 + dx + 1) * 128]
                            nc.tensor.matmul(ps, lhsT, rhs, start=first,
                                             stop=(kt == kt_n - 1 and dy == 2 and dx == 2))
                            first = False
                return ps

            # ---- attention apply interleaved with SA conv1 (keeps PE fed
            # while vector drains assembles); per-ct fp8 cast as soon as both
            # heads of a ct block are assembled ----
            sa1slabs = stream_sa(w['sa1wT'])
            sa1_jobs = [(mt, chunk) for mt in range(2) for chunk in range(NCHUNK)]
            attn_nop = [ct_t[:].rearrange("p (y x) -> p y x", x=40) for ct_t in attn]
            for h in range(8):
                expT, rbc = recs[h]
                o = (h % 2) * 64
                for py in range(4):
                    psy = psY.tile([128, 512], F32, tag="aps", name="aps")
                    for px in range(4):
                        pp = py * 4 + px
                        lhsT = v_ct[h // 2][0:100, pp * 128 + o: pp * 128 + o + 64]
                        nc.tensor.matmul(psy[o:o + 64, px * 100:px * 100 + 100], lhsT, expT[:],
                                         start=(px == 0), stop=(px == 3))
                    dstg = attn_nop[h // 2][o:o + 64, py:40:4, :]
                    dstg = dstg.rearrange("p a (pwx px) -> p a pwx px", px=4)
                    in0 = psy[o:o + 64, 0:400].rearrange("p (px phy pwx) -> p phy pwx px",
                                                         px=4, phy=10)
                    in1 = rbc[o:o + 64].rearrange("p (a b) -> p a b", b=10)
                    in1 = in1.unsqueeze(3).broadcast_to([64, 10, 10, 4])
                    nc.vector.scalar_tensor_tensor(dstg, in0, 0.0, in1,
                                                   AluOpType.bypass, AluOpType.mult)
                if h % 2 == 1:
                    ct = h // 2
                    dst8 = attn8v[:, ct, 43:43 + PADPIX].rearrange("p (y x) -> p y x", x=42)
                    nc.vector.tensor_copy(dst8[:, 1:41, 1:41], attn_nop[ct])
                # one SA1 (mt, chunk) group between heads keeps the PE busy
                mt, chunk = sa1_jobs[h]
                ps = conv3x3(xca, sa1slabs, mt, 2, chunk)[:, 0:CHUNK_N]
                y0 = chunk * CHUNK_ROWS
                dst = xsa1[mt][:].rearrange("p (y x) -> p y x", x=WP)[:, 1 + y0:11 + y0, 1:41]
                nc.scalar.activation(dst, ps.rearrange("p (a b) -> p a b", b=40), AF.Silu,
                                     bias=vs['sa1b'][:, mt:mt + 1], scale=vs['sa1s'][:, mt:mt + 1])

            # ---- SA conv2 + residual ----
            sa2slabs = stream_sa(w['sa2wT'])
            for mt in range(2):
                for chunk in range(NCHUNK):
                    ps = conv3x3(xsa1, sa2slabs, mt, 2, chunk)[:, 0:CHUNK_N]
                    y0 = chunk * CHUNK_ROWS
                    tsilu = mp.tile([128, CHUNK_N], F32, tag="silu", bufs=3)
                    nc.scalar.activation(tsilu[:], ps, AF.Silu,
                                         bias=vs['sa2b'][:, mt:mt + 1], scale=vs['sa2s'][:, mt:mt + 1])
                    xc = xca[mt][:].rearrange("p (y x) -> p y x", x=WP)[:, 1 + y0:11 + y0, 1:41]
                    nc.gpsimd.tensor_tensor(a2[mt][:, y0 * 40:(y0 + 10) * 40],
                                            tsilu[:].rearrange("p (a b) -> p a b", b=40),
                                            xc, AluOpType.add)
            if 'a2' in taps:
                for i in range(2):
                    af = mp.tile([128, 1600], F32, tag="tapg", bufs=1)
                    nc.vector.tensor_copy(af[:], a2[i][:])
                    nc.sync.dma_start(out=taps['a2'][s, i], in_=af[:])

            # ---- conv3 (fp8 DoubleRow) + residual -> x2_out ----
            for mt in range(4):
                for chunk in range(NCHUNK):
                    ps0 = psC.tile([128, 512], F32, tag="cps", name="cps")
                    ps = ps0[:, 0:420]
                    y0 = chunk * CHUNK_ROWS
                    first = True
                    for g in range(2):
                        wslab = c3w8_sb[:, (g * 4 + mt) * 2304:(g * 4 + mt + 1) * 2304] \
                            .rearrange("p (k two m) -> p k two m", two=2, m=128)
                        for dy in range(3):
                            for dx in range(3):
                                s0 = 42 + (y0 + dy) * 42 + dx
                                rhs = attn8v[:, 2 * g:2 * g + 2, s0:s0 + 420]
                                nc.tensor.matmul(ps, wslab[:, dy * 3 + dx], rhs,
                                                 start=first,
                                                 stop=(g == 1 and dy == 2 and dx == 2),
                                                 perf_mode=mybir.MatmulPerfMode.DoubleRow)
                                first = False
                    tsilu = mp.tile([128, CHUNK_N], F32, tag="silu", bufs=3)
                    psv = ps.rearrange("p (y x) -> p y x", x=42)[:, :, 1:41]
                    nc.scalar.activation(tsilu[:].rearrange("p (a b) -> p a b", b=40),
                                         psv, AF.Silu,
                                         bias=vs['c3b'][:, mt:mt + 1], scale=vs['c3s'][:, mt:mt + 1])
                    osb = mp.tile([128, CHUNK_N], F32, tag="osb", bufs=4)
                    at2 = attn_nop[mt][:, y0:y0 + 10, :]
                    nc.gpsimd.tensor_tensor(osb[:].rearrange("p (a b) -> p a b", b=40),
                                            tsilu[:].rearrange("p (a b) -> p a b", b=40),
                                            at2, AluOpType.add)
                    nc.sync.dma_start(
                        out=out.ap()[s, 512 + mt * 128:512 + (mt + 1) * 128]
                            .rearrange("p y x -> p (y x)")[:, y0 * 40:(y0 + 10) * 40],
                        in_=osb[:])

            # ---- conv2 (1x1) -> x1_out ----
            for mt in range(4):
                for chunk in range(NCHUNK):
                    # ps layout: col = r*200 + y*40 + x  (pixel (2y+r)*40+x)
                    ps0 = psC.tile([128, 512], F32, tag="cps", name="cps")
                    ps = ps0[:, 0:CHUNK_N]
                    y0 = chunk * CHUNK_ROWS
                    first = True
                    for r in range(2):
                        for kt in range(4):
                            nc.tensor.matmul(
                                ps[:, r * 200:(r + 1) * 200],
                                c2_sb[:, (kt * 4 + mt) * 128:(kt * 4 + mt + 1) * 128],
                                up_ap(kt, s, chunk), start=first, stop=False)
                            first = False
                    for i in range(2):
                        kt = 4 + i
                        rhs = a2[i][:, y0 * 40:(y0 + 10) * 40]
                        rhs = rhs.rearrange("p (y r x) -> p r y x", y=5, r=2)
                        nc.tensor.matmul(ps, c2_sb[:, (kt * 4 + mt) * 128:(kt * 4 + mt + 1) * 128],
                                         rhs, start=False, stop=(i == 1))
                    osb = mp.tile([128, CHUNK_N], F32, tag="osb", bufs=4)
                    oview = osb[:].rearrange("p (y r x) -> p y r x", y=5, r=2)
                    pv = ps.rearrange("p (r y x) -> p y r x", r=2, y=5)
                    nc.scalar.activation(oview, pv, AF.Silu,
                                         bias=vs['c2b'][:, mt:mt + 1], scale=vs['c2s'][:, mt:mt + 1])
                    nc.sync.dma_start(
                        out=out.ap()[s, mt * 128:(mt + 1) * 128]
                            .rearrange("p y x -> p (y x)")[:, y0 * 40:(y0 + 10) * 40],
                        in_=osb[:])

